# revision 61
# baseline (speedup 1.0000x reference)
"""Trainium2 Bass kernel for MQA sliding-window causal self-attention.

Sharding: 8 cores = DP(batch=2) x TP(head-groups=4). Each core computes 4 of
16 query heads for one batch element, shared KV head replicated. Host
pre-packs transposed/padded bf16 layouts (Wk pre-scaled by 1/8 so the
softmax scale folds into k's rmsnorm term); gathers + sums the 4 TP partial
outputs per batch element.

v2 vs baseline (152349ns):
  - ONE act table for the whole kernel: rstd = exp(-0.5*ln(ssum)) via the
    natural_log_exp_and_others set (act-table monkeypatch strips ln/exp from
    other sets so the greedy CFG pass can't thrash Sqrt<->Exp tables: was 11
    LoadActFuncSet = 14.1us on Act + PE stalls).
  - startup: wq + first x group split into 8 per-k8 DMA slices on two HWDGE
    queues so the first projection matmul issues at ~0.6us (was 7.9us gap).
  - stage_b PSUM->SBUF copies + y output copies moved Act->Pool (gpsimd);
    square moved Act->DVE (bf16 2x self-mul); Act runs ~only exp.
  - edge masks gpsimd->DVE (bf16 2x) and merged across the 2 head-pair sets
    (one et tile [128,2ps,2,1152] per j-block).
  - epilogue: softmax divide writes aoT halves directly (PSUM in0 allows
    out-partition-base offset), killing the aop intermediate, its copies,
    and the odd-half SBUF DMA.
"""
import numpy as np
import ml_dtypes
from contextlib import ExitStack

import concourse.bass as bass
import concourse.tile as tile
import concourse.mybir as mybir
from concourse import bacc
from concourse.bass_utils import run_bass_kernel_spmd

# ---- act-table selection patch: keep ln/exp ONLY in the one set that has
# both, so the greedy table-load pass emits a single LoadActFuncSet ----
import concourse.hw_specs as _hs
import concourse.bacc as _bacc_mod

_ONLY_SET = "natural_log_exp_and_others"
_orig_get_tables = _hs.get_activation_tables


def _patched_tables(arch):
    tabs = dict(_orig_get_tables(arch))
    keep = tabs[_ONLY_SET]
    return {k: (v if k == _ONLY_SET else (v - keep)) for k, v in tabs.items()}


_bacc_mod.get_activation_tables = _patched_tables

F32 = mybir.dt.float32
BF = mybir.dt.bfloat16
AF = mybir.ActivationFunctionType
ALU = mybir.AluOpType
BF_NP = ml_dtypes.bfloat16

B, S, E, H, KV, D = 2, 2048, 1024, 16, 1, 64
HALF = D // 2
GATE_CH = 32
WIN = 1024
NCORES = 8
TP = 4
HPC = H // TP            # heads per core = 4
HD = HPC * D             # per-core q width = 256
SB = S // 128            # 16 s-blocks
WB = WIN // 128          # 8 window blocks
QKW = HD + 2 * D + 1     # 385 (q 256 | k 64 | v 64 | gate 1)
RW = HD + D              # 320 roped width (4 q heads + k)
QNW = HD + 2 * D         # 384 qn width (q 256 | k | k-dup)
GW = 2                   # s-blocks per phase-1 group
NG = SB // GW            # 8 groups

TRACE = False
LAST_RESULT = [None]
_NC_CACHE = [None]


def _build():
    nc = bacc.Bacc()

    xg2 = nc.dram_tensor("xg2", [NG, 128, 8 * 256], BF, kind="ExternalInput")
    wqg = nc.dram_tensor("wqg", [128, 8 * QKW], BF, kind="ExternalInput")
    cs2 = nc.dram_tensor("cs2", [128, SB * 128], BF, kind="ExternalInput")
    ve2 = nc.dram_tensor("ve2", [128, SB * 64], BF, kind="ExternalInput")
    wo2 = nc.dram_tensor("wo2", [2, 128, E], BF, kind="ExternalInput")
    mkd = nc.dram_tensor("mkd", [128, 2 * 128], BF, kind="ExternalInput")
    mkf = nc.dram_tensor("mkf", [128, 2 * 128], BF, kind="ExternalInput")
    y = nc.dram_tensor("y", [SB, 128, E], BF, kind="ExternalOutput")

    with tile.TileContext(nc) as tc, ExitStack() as top:
        const = top.enter_context(tc.tile_pool(name="const", bufs=1))
        persist = top.enter_context(tc.tile_pool(name="persist", bufs=1))

        # ---- persistent activations ----
        qkT = [persist.tile([128, 3, GW * 128], BF, name=f"qkT{g}")
               for g in range(NG)]
        aoT = [persist.tile([128, S], BF, name=f"aoT{i}") for i in range(2)]
        vex = [persist.tile([128, 128], BF, name=f"vex{s}")
               for s in range(SB)]
        for s in range(SB):
            nc.gpsimd.memset(vex[s][:, 64:128], 1.0)
        # rstd per group: [:, :, 0:HPC] = q heads, [:, :, HPC] = k
        rg = [persist.tile([128, GW, 5], F32, name=f"rg{g}")
              for g in range(NG)]
        gate_sb = persist.tile([128, SB], F32)
        ge = persist.tile([128, SB], F32)
        gd = persist.tile([128, SB], F32)
        sig = persist.tile([128, SB], F32)

        maskd_sb = const.tile([128, 2, 128], BF)
        maskf_sb = const.tile([128, 2, 128], BF)
        cs_sb = const.tile([128, SB, 128], BF)
        ve_sb = const.tile([128, SB, 64], BF)
        wo_sb = [const.tile([128, E], BF, name=f"wo{i}") for i in range(2)]
        # weights in 2+2+4 k8-chunks: HWDGE costs ~625ns per DMA regardless
        # of size, so keep the count low, but quarter the first chunks so
        # the opening projection can issue ~1.5us sooner
        WQP = [(0, 2), (2, 2), (4, 4)]
        wq_h = [const.tile([128, n, QKW], BF, name=f"wqh{i}")
                for i, (k0, n) in enumerate(WQP)]

        with ExitStack() as p1:
            xpool = p1.enter_context(tc.tile_pool(name="xg", bufs=1))
            work = p1.enter_context(tc.tile_pool(name="work", bufs=1))
            big_psp = p1.enter_context(tc.tile_pool(name="big_ps", bufs=1,
                                                    space="PSUM"))
            strip_psp = p1.enter_context(tc.tile_pool(name="strip", bufs=1,
                                                      space="PSUM"))
            acc_psp = p1.enter_context(tc.tile_pool(name="acc", bufs=1,
                                                    space="PSUM"))
            expp = p1.enter_context(tc.tile_pool(name="expp", bufs=1))
            ep = p1.enter_context(tc.tile_pool(name="ep", bufs=1))
            yp = p1.enter_context(tc.tile_pool(name="yp", bufs=1))

            xg = {}
            st = {}

            # group-0 x chunks interleaved with the weight chunks on two
            # HWDGE queues
            xg0_h = [xpool.tile([128, n, 256], BF, tag=f"xg0{i}",
                                name=f"xg0h{i}", bufs=1)
                     for i, (k0, n) in enumerate(WQP)]
            for i, (k0, n) in enumerate(WQP):
                nc.sync.dma_start(
                    wq_h[i], wqg[:, k0 * QKW:(k0 + n) * QKW]
                    .rearrange("p (k c) -> p k c", k=n))
                nc.scalar.dma_start(
                    xg0_h[i], xg2[0, :, k0 * 256:(k0 + n) * 256]
                    .rearrange("p (k c) -> p k c", k=n))

            def wq_part(k8):
                i = 0 if k8 < 2 else (1 if k8 < 4 else 2)
                return wq_h[i][:, k8 - WQP[i][0], :]

            def xg0_part(k8):
                i = 0 if k8 < 2 else (1 if k8 < 4 else 2)
                return xg0_h[i][:, k8 - WQP[i][0], :]

            def load_group(g):
                t = xpool.tile([128, 8, 256], BF, tag="xg", name="xg_t", bufs=5)
                nc.sync.dma_start(
                    t, xg2[g, :, :].rearrange("p (k c) -> p k c", k=8))
                xg[g] = t

            load_group(1)
            # remaining constants on the Act HWDGE queue, behind nothing hot
            nc.scalar.dma_start(cs_sb,
                                cs2[:, :].rearrange("p (s c) -> p s c", s=SB))
            nc.scalar.dma_start(ve_sb,
                                ve2[:, :].rearrange("p (s c) -> p s c", s=SB))
            nc.scalar.dma_start(maskd_sb,
                                mkd[:, :].rearrange("p (h x) -> p h x", h=2))
            nc.scalar.dma_start(maskf_sb,
                                mkf[:, :].rearrange("p (h x) -> p h x", h=2))
            for i in range(2):
                nc.scalar.dma_start(wo_sb[i], wo2[i, :, :])

            def stage_a(g):
                # rotate the projection outputs through ALL THREE psum tags
                # (their slots are the same 2KB size; strips/acc are idle
                # until the attention loop) so the projections aren't
                # throttled by the 2-deep big-tag <-> stage_b round-trip
                r = g % 3
                if r == 1:
                    stp = strip_psp.tile([128, 2, 512], F32, tag="strip",
                                         name="strip", bufs=2)
                    ps_pair = [stp[:, li, 0:QKW] for li in range(GW)]
                elif r == 2:
                    ps_pair = []
                    for li in range(GW):
                        at = acc_psp.tile([128, 2, 256], F32, tag="acc",
                                          name="acc", bufs=2)
                        ps_pair.append(
                            at[:].rearrange("p a b -> p (a b)")[:, 0:QKW])
                else:
                    ps_pair = []
                    for li in range(GW):
                        big = big_psp.tile([128, 512], F32, tag="big",
                                           name="big_ps", bufs=2)
                        ps_pair.append(big[:, 0:QKW])
                for li in range(GW):
                    lcol = slice(li * 128, (li + 1) * 128)
                    for k8 in range(8):
                        xs = (xg0_part(k8)[:, lcol] if g == 0
                              else xg[g][:, k8, lcol])
                        nc.tensor.matmul(ps_pair[li], xs, wq_part(k8),
                                         start=(k8 == 0), stop=(k8 == 7),
                                         skip_group_check=True)
                st[g] = dict(ps=ps_pair)

            def stage_b(g):
                # one wide PSUM->SBUF copy per s-block (frees the big PSUM
                # buf sooner for the next projection); v/gate extracted from
                # SBUF on Pool, off the critical chain
                s_ = st[g]
                qkvsb = work.tile([128, GW, QKW], BF, tag="qkvsb",
                                  name="qkvsb", bufs=3)
                for li in range(GW):
                    ps_t = s_["ps"][li]
                    nc.scalar.copy(qkvsb[:, li, :], ps_t)
                for li in range(GW):
                    sb = g * GW + li
                    nc.gpsimd.tensor_copy(vex[sb][:, 0:64],
                                          qkvsb[:, li, RW:RW + 64])
                    nc.gpsimd.tensor_copy(gate_sb[:, sb:sb + 1],
                                          qkvsb[:, li, RW + 64:QKW])
                s_["qkvsb"] = qkvsb
                del s_["ps"]

            def csbc(g, off, width):
                # [128, li(2), 5-head bcast, width] view of cos/sin table
                return bass.AP(tensor=cs_sb.tensor,
                               offset=cs_sb.offset + (g * GW) * 128 + off,
                               ap=[list(cs_sb.ap[0]), [128, GW], [0, 5],
                                   [1, width]])

            def stage_c(g):
                s_ = st[g]
                qsb = s_["qkvsb"]
                qk5 = bass.AP(tensor=qsb.tensor, offset=qsb.offset,
                              ap=[list(qsb.ap[0]), [QKW, GW], [D, 5], [1, D]])
                tm1 = work.tile([128, GW, RW], BF, tag="tm1", name="tm1",
                                bufs=3)
                tm2 = work.tile([128, GW, RW], BF, tag="tm2", name="tm2",
                                bufs=3)
                tm1v = tm1[:].rearrange("p l (h d) -> p l h d", h=5)
                tm2v = tm2[:].rearrange("p l (h d) -> p l h d", h=5)
                nc.vector.tensor_mul(tm1v, qk5, csbc(g, 0, D))
                nc.gpsimd.tensor_mul(tm2v[:, :, :, 0:HALF],
                                     qk5[:, :, :, HALF:D], csbc(g, D, HALF))
                nc.vector.tensor_mul(tm2v[:, :, :, HALF:D],
                                     qk5[:, :, :, 0:HALF],
                                     csbc(g, D + HALF, HALF))
                qk_r = work.tile([128, GW, RW], BF, tag="qkr", name="qk_r",
                                 bufs=4)
                nc.vector.tensor_add(qk_r, tm1, tm2)
                s_["qk_r"] = qk_r

            def stage_d(g):
                s_ = st[g]
                qk_r = s_["qk_r"]
                sq = work.tile([128, GW, RW], BF, tag="sq", name="sq", bufs=3)
                nc.vector.tensor_mul(sq, qk_r, qk_r)
                ssum = work.tile([128, GW, 5], BF, tag="ssum", name="ssum",
                                 bufs=3)
                with nc.allow_low_precision(reason="rmsnorm ssum bf16: "
                                            "0.4% on ssum -> 0.2% on rstd"):
                    nc.vector.reduce_sum(
                        ssum[:].rearrange("p a b -> p (a b)"),
                        sq[:].rearrange("p l (h d) -> p (l h) d", h=5),
                        axis=mybir.AxisListType.X)
                # rstd_q = exp(-0.5*ln(ssum)); rstd_k = exp(-0.5*ln(ssum/64))
                # = sqrt(64/ssum) -- the D-scale enters via the Ln input
                # scale (rmsnorm is scale-invariant, so it can't come from a
                # host-side Wk scale)
                lt = work.tile([128, GW, 5], F32, tag="lt", name="lt", bufs=3)
                nc.scalar.activation(lt[:, :, 0:HPC], ssum[:, :, 0:HPC],
                                     AF.Ln, bias=0.0, scale=1.0)
                nc.scalar.activation(lt[:, :, HPC:5], ssum[:, :, HPC:5],
                                     AF.Ln, bias=0.0, scale=1.0 / D)
                nc.scalar.activation(rg[g][:], lt, AF.Exp, bias=0.0,
                                     scale=-0.5)

            def stage_e(g):
                s_ = st[g]
                qn = work.tile([128, GW, QNW], BF, tag="qn", name="qn", bufs=4)
                rbc = bass.AP(tensor=rg[g].tensor, offset=rg[g].offset,
                              ap=[list(rg[g].ap[0]), [5, GW], [1, HPC],
                                  [0, D]])
                nc.vector.tensor_mul(
                    qn[:, :, 0:HD].rearrange("p l (h d) -> p l h d", h=HPC),
                    s_["qk_r"][:, :, 0:HD].rearrange("p l (h d) -> p l h d",
                                                     h=HPC),
                    rbc)
                # roped k (unnormalized, pre-scaled 1/8), duplicated twice
                kin = bass.AP(tensor=s_["qk_r"].tensor,
                              offset=s_["qk_r"].offset + HD,
                              ap=[list(s_["qk_r"].ap[0]), [RW, GW], [0, 2],
                                  [1, D]])
                kout = bass.AP(tensor=qn.tensor, offset=qn.offset + HD,
                               ap=[list(qn.ap[0]), [QNW, GW], [D, 2], [1, D]])
                nc.vector.tensor_copy(kout, kin)
                s_["qn"] = qn

            def stage_f(g):
                s_ = st.pop(g)
                qn = s_["qn"]
                for li in range(GW):
                    lcol = slice(li * 128, (li + 1) * 128)
                    nc.sync.dma_start(qkT[g][:, :, lcol], qn[:, li, :],
                                      transpose=True)

            def gate_group(g):
                # per-group gate: spreads the sigmoid + vex gating over the
                # phase-1 ticks instead of a 16-op DVE burst at t=NG
                gcol = slice(g * GW, (g + 1) * GW)
                nc.scalar.activation(ge[:, gcol], gate_sb[:, gcol], AF.Exp,
                                     bias=0.0, scale=-1.0)
                nc.vector.tensor_scalar_add(gd[:, gcol], ge[:, gcol], 1.0)
                nc.vector.reciprocal(sig[:, gcol], gd[:, gcol])
                for sb in range(g * GW, (g + 1) * GW):
                    nc.vector.scalar_tensor_tensor(
                        out=vex[sb][:, 0:64], in0=ve_sb[:, sb, :],
                        scalar=sig[:, sb:sb + 1], in1=vex[sb][:, 0:64],
                        op0=ALU.mult, op1=ALU.add)

            # ---------------- attention j-step pieces ----------------
            exps = {0: {}, 1: {}}
            acst = {}

            def mm1(ps, j):
                nq = min(j + WB + 1, SB) - j
                et = expp.tile([128, 2, (WB + 1) * 128], BF, tag=f"exp{ps}",
                               name=f"exp{ps}", bufs=10)
                exps[ps][j] = et
                kt = qkT[j // 2]
                jcol = slice((j % 2) * 128, (j % 2) * 128 + 128)
                rk_ap = rg[j // 2][:, (j % 2), HPC:5]
                # group-aligned segments: [1] if j odd, then pairs, tail [1]
                segs = []
                b = j
                if b % 2 == 1:
                    segs.append((b, 1))
                    b += 1
                while b + 1 < j + nq:
                    segs.append((b, 2))
                    b += 2
                if b < j + nq:
                    segs.append((b, 1))
                # pack segments into strip buffers of <= 4 blocks, exp per buf
                si = 0
                off = 0
                while si < len(segs):
                    take = []
                    blk = 0
                    while si < len(segs) and blk + segs[si][1] <= 4:
                        take.append(segs[si])
                        blk += segs[si][1]
                        si += 1
                    stp = strip_psp.tile([128, 2, 512], F32, tag="strip",
                                         name="strip", bufs=2)
                    co = 0
                    for b0, nb in take:
                        w = nb * 128
                        g0 = b0 // 2
                        qcol = slice((b0 % 2) * 128, (b0 % 2) * 128 + w)
                        nc.tensor.matmul(stp[:, 0, co:co + w],
                                         kt[0:64, 2, jcol],
                                         qkT[g0][0:64, ps, qcol],
                                         start=True, stop=True,
                                         skip_group_check=True)
                        nc.tensor.matmul(stp[:, 1, co:co + w],
                                         kt[64:128, 2, jcol],
                                         qkT[g0][64:128, ps, qcol],
                                         start=True, stop=True,
                                         skip_group_check=True)
                        co += w
                    cw = blk * 128
                    nc.scalar.activation(et[:, :, off:off + cw],
                                         stp[:, :, 0:cw], AF.Exp,
                                         bias=0.0, scale=rk_ap)
                    if off == 0:
                        nc.vector.tensor_mul(et[:, :, 0:128], et[:, :, 0:128],
                                             maskd_sb)
                    off += cw
                if nq == WB + 1:
                    fcol = slice(WB * 128, (WB + 1) * 128)
                    nc.vector.tensor_mul(et[:, :, fcol], et[:, :, fcol],
                                         maskf_sb)

            def mm2pair(ps, m):
                q0, q1 = 2 * m, 2 * m + 1
                a = acc_psp.tile([128, 2, 256], F32, tag="acc", name="acc",
                                 bufs=2)
                first = True
                if q0 - WB >= 0:
                    jj = q0 - WB
                    o0 = (q0 - jj) * 128
                    nc.tensor.matmul(
                        a[:, :, 0:128], vex[jj][:],
                        exps[ps][jj][:, :, o0:o0 + 128],
                        start=True, stop=False, skip_group_check=True)
                    first = False
                for jj in range(max(0, q1 - WB), q0 + 1):
                    off = (q0 - jj) * 128
                    nc.tensor.matmul(a, vex[jj][:],
                                     exps[ps][jj][:, :, off:off + 256],
                                     start=first, stop=False,
                                     skip_group_check=True)
                    first = False
                nc.tensor.matmul(a[:, :, 128:256], vex[q1][:],
                                 exps[ps][q1][:, :, 0:128],
                                 start=False, stop=True,
                                 skip_group_check=True)
                acst[(ps, m)] = dict(a=a)

            def epi1(ps, m):
                s_ = acst[(ps, m)]
                rec = ep.tile([64, 2, 256], BF, tag=f"rec{ps}",
                              name=f"rec{ps}", bufs=2)
                with nc.allow_low_precision(reason="softmax denom recip"):
                    nc.vector.reciprocal(rec, s_["a"][64:128, :, :])
                s_["rec"] = rec

            def epi2(ps, m):
                s_ = acst.pop((ps, m))
                a, rec = s_["a"], s_["rec"]
                scol = slice(2 * m * 128, (2 * m + 2) * 128)
                # write aoT halves straight from PSUM x rec (PSUM in0 lets
                # the out partition base differ from the input bases)
                nc.vector.tensor_mul(aoT[ps][0:64, scol], a[0:64, 0, :],
                                     rec[:, 0, :])
                nc.vector.tensor_mul(aoT[ps][64:128, scol], a[0:64, 1, :],
                                     rec[:, 1, :])

            def mm3(sb, y_t, half, tail=False):
                scol = slice(sb * 128, (sb + 1) * 128)
                for nch in range(2):
                    y_ps = big_psp.tile([128, 512], F32, tag="big",
                                        name="y_ps", bufs=2)
                    for i in range(2):
                        nc.tensor.matmul(y_ps, aoT[i][:, scol],
                                         wo_sb[i][:, nch * 512:(nch + 1) * 512],
                                         start=(i == 0), stop=(i == 1),
                                         skip_group_check=True)
                    # tail: Act is idle after the last exp, so split the
                    # final copies across both engines to shorten the drain
                    eng = nc.scalar.copy if (tail and nch == 1) else \
                        nc.vector.tensor_copy
                    eng(y_t[:, half, nch * 512:(nch + 1) * 512], y_ps)

            yts = {}

            def jblock_mm1(j):
                if j < SB:
                    mm1(0, j)
                    mm1(1, j)

            def jblock(j):
                # mm3 for the pair m_ is split across this odd tick and the
                # following even one: 2 y_ps PSUM allocs per tick instead of
                # 4 keeps the DVE y-copy off mm3's big-tag rotation wait
                if j % 2 == 1:
                    if j >= 3 and (j - 3) // 2 < WB:
                        m_ = (j - 3) // 2
                        for ps in range(2):
                            epi2(ps, m_)
                        y_t = yp.tile([128, 2, E], BF, tag="ysb", name="y_t",
                                      bufs=2)
                        if m_ == WB - 1:
                            # final pair: both halves now, split DMAs, so the
                            # drain isn't gated on a whole extra tick
                            mm3(2 * m_, y_t, 0, tail=True)
                            nc.sync.dma_start(
                                y[2 * m_:2 * m_ + 1, :, :]
                                .rearrange("s p e -> p s e"),
                                y_t[:, 0:1, :])
                            mm3(2 * m_ + 1, y_t, 1, tail=(m_ >= 3))
                            nc.sync.dma_start(
                                y[2 * m_ + 1:2 * m_ + 2, :, :]
                                .rearrange("s p e -> p s e"),
                                y_t[:, 1:2, :])
                        else:
                            yts[m_] = y_t
                            mm3(2 * m_, y_t, 0, tail=(m_ >= 3))
                    if (j - 1) // 2 < WB:
                        m = (j - 1) // 2
                        mm2pair(0, m)
                        mm2pair(1, m)
                else:
                    if j >= 4 and (j - 4) // 2 < WB - 1:
                        m_ = (j - 4) // 2
                        y_t = yts.pop(m_)
                        mm3(2 * m_ + 1, y_t, 1, tail=(m_ >= 3))
                        nc.sync.dma_start(
                            y[2 * m_:2 * m_ + 2, :, :]
                            .rearrange("s p e -> p s e"), y_t)
                    if j >= 2 and j // 2 - 1 < WB:
                        for ps in range(2):
                            epi1(ps, j // 2 - 1)

            # ---------------- merged tick loop ----------------
            for t in range(18):
                if t + 2 < NG:
                    load_group(t + 2)
                if 0 <= t - 3 < NG:
                    stage_f(t - 3)
                if 0 <= t - 1 < NG:
                    stage_b(t - 1)
                if 0 <= t - 2 < NG:
                    stage_d(t - 2)
                    stage_e(t - 2)
                if 0 <= t - 1 < NG:
                    stage_c(t - 1)
                if 0 <= t - 3 < NG:
                    gate_group(t - 3)
                if t < NG:
                    stage_a(t)
                # both js' score strips first: Act's exp queue stays a
                # half-tick ahead of the mm2 reads of the fresh et tiles
                for j in (2 * (t - 7) - 1, 2 * (t - 7)):
                    if 0 <= j < SB + 2:
                        jblock_mm1(j)
                for j in (2 * (t - 7) - 1, 2 * (t - 7)):
                    if 0 <= j < SB + 2:
                        jblock(j)

    nc.compile()
    return nc


def _prep_core_inputs(c, x, ve, cos, sin, Wq, Wk, Wv, Wo, Wg):
    b = c // TP
    h0 = (c % TP) * HD
    xT = np.ascontiguousarray(x[b].T).astype(BF_NP)          # [E, S]
    xg2 = np.empty((NG, 128, 8 * 256), BF_NP)
    for g in range(NG):
        for k8 in range(8):
            xg2[g, :, k8 * 256:(k8 + 1) * 256] = \
                xT[k8 * 128:(k8 + 1) * 128, g * 256:(g + 1) * 256]
    wg_pad = np.zeros((E, 1), np.float32)
    wg_pad[:GATE_CH, 0] = Wg[:, 0]
    wqkv = np.concatenate([Wq[:, h0:h0 + HD], Wk, Wv, wg_pad], axis=1)
    wqg = np.ascontiguousarray(
        wqkv.reshape(8, 128, QKW).transpose(1, 0, 2)
        .reshape(128, 8 * QKW)).astype(BF_NP)
    ccss = np.concatenate([cos, cos, sin, -sin], axis=1)     # [S, 128]
    cs2 = np.ascontiguousarray(
        ccss.reshape(SB, 128, 128).transpose(1, 0, 2).reshape(128, SB * 128)
    ).astype(BF_NP)
    ve2 = np.ascontiguousarray(
        (2.0 * ve[b]).reshape(SB, 128, 64).transpose(1, 0, 2)
        .reshape(128, SB * 64)).astype(BF_NP)
    wo2 = np.ascontiguousarray(
        Wo[h0:h0 + HD, :].reshape(2, 128, E)).astype(BF_NP)
    ii = np.arange(128)
    md = (ii[None, :] >= ii[:, None]).astype(np.float32)     # [ki, qi]
    mf = 1.0 - md
    mkd = np.tile(md, (1, 2)).reshape(128, 256).astype(BF_NP)
    mkf = np.tile(mf, (1, 2)).reshape(128, 256).astype(BF_NP)
    return dict(xg2=xg2, wqg=wqg, cs2=cs2, ve2=ve2, wo2=wo2,
                mkd=mkd, mkf=mkf)


def kernel(x, ve, cos, sin, Wq, Wk, Wv, Wo, Wg, window_size):
    assert int(window_size) == WIN
    x = np.asarray(x, np.float32)
    ve = np.asarray(ve, np.float32)
    cos = np.asarray(cos, np.float32)
    sin = np.asarray(sin, np.float32)
    Wq = np.asarray(Wq, np.float32)
    Wk = np.asarray(Wk, np.float32)
    Wv = np.asarray(Wv, np.float32)
    Wo = np.asarray(Wo, np.float32)
    Wg = np.asarray(Wg, np.float32)

    if _NC_CACHE[0] is None:
        _NC_CACHE[0] = _build()
    nc = _NC_CACHE[0]

    in_maps = [_prep_core_inputs(c, x, ve, cos, sin, Wq, Wk, Wv, Wo, Wg)
               for c in range(NCORES)]
    res = run_bass_kernel_spmd(nc, in_maps, core_ids=list(range(NCORES)),
                               trace=TRACE)
    LAST_RESULT[0] = res

    out = np.zeros((B, S, E), np.float32)
    for c in range(NCORES):
        out[c // TP] += res.results[c]["y"].astype(np.float32).reshape(S, E)
    return out


# revision 64
# speedup vs baseline: 1.0017x; 1.0017x over previous
"""Trainium2 Bass kernel for MQA sliding-window causal self-attention.

Sharding: 8 cores = DP(batch=2) x TP(head-groups=4). Each core computes 4 of
16 query heads for one batch element, shared KV head replicated. Host
pre-packs transposed/padded bf16 layouts (Wk pre-scaled by 1/8 so the
softmax scale folds into k's rmsnorm term); gathers + sums the 4 TP partial
outputs per batch element.

v2 vs baseline (152349ns):
  - ONE act table for the whole kernel: rstd = exp(-0.5*ln(ssum)) via the
    natural_log_exp_and_others set (act-table monkeypatch strips ln/exp from
    other sets so the greedy CFG pass can't thrash Sqrt<->Exp tables: was 11
    LoadActFuncSet = 14.1us on Act + PE stalls).
  - startup: wq + first x group split into 8 per-k8 DMA slices on two HWDGE
    queues so the first projection matmul issues at ~0.6us (was 7.9us gap).
  - stage_b PSUM->SBUF copies + y output copies moved Act->Pool (gpsimd);
    square moved Act->DVE (bf16 2x self-mul); Act runs ~only exp.
  - edge masks gpsimd->DVE (bf16 2x) and merged across the 2 head-pair sets
    (one et tile [128,2ps,2,1152] per j-block).
  - epilogue: softmax divide writes aoT halves directly (PSUM in0 allows
    out-partition-base offset), killing the aop intermediate, its copies,
    and the odd-half SBUF DMA.
"""
import numpy as np
import ml_dtypes
from contextlib import ExitStack

import concourse.bass as bass
import concourse.tile as tile
import concourse.mybir as mybir
from concourse import bacc
from concourse.bass_utils import run_bass_kernel_spmd

# ---- act-table selection patch: keep ln/exp ONLY in the one set that has
# both, so the greedy table-load pass emits a single LoadActFuncSet ----
import concourse.hw_specs as _hs
import concourse.bacc as _bacc_mod

_ONLY_SET = "natural_log_exp_and_others"
_orig_get_tables = _hs.get_activation_tables


def _patched_tables(arch):
    tabs = dict(_orig_get_tables(arch))
    keep = tabs[_ONLY_SET]
    return {k: (v if k == _ONLY_SET else (v - keep)) for k, v in tabs.items()}


_bacc_mod.get_activation_tables = _patched_tables

F32 = mybir.dt.float32
BF = mybir.dt.bfloat16
AF = mybir.ActivationFunctionType
ALU = mybir.AluOpType
BF_NP = ml_dtypes.bfloat16

B, S, E, H, KV, D = 2, 2048, 1024, 16, 1, 64
HALF = D // 2
GATE_CH = 32
WIN = 1024
NCORES = 8
TP = 4
HPC = H // TP            # heads per core = 4
HD = HPC * D             # per-core q width = 256
SB = S // 128            # 16 s-blocks
WB = WIN // 128          # 8 window blocks
QKW = HD + 2 * D + 1     # 385 (q 256 | k 64 | v 64 | gate 1)
RW = HD + D              # 320 roped width (4 q heads + k)
QNW = HD + 2 * D         # 384 qn width (q 256 | k | k-dup)
GW = 2                   # s-blocks per phase-1 group
NG = SB // GW            # 8 groups

TRACE = False
LAST_RESULT = [None]
_NC_CACHE = [None]


def _build():
    nc = bacc.Bacc()

    xg2 = nc.dram_tensor("xg2", [NG, 128, 8 * 256], BF, kind="ExternalInput")
    wqg = nc.dram_tensor("wqg", [128, 8 * QKW], BF, kind="ExternalInput")
    cs2 = nc.dram_tensor("cs2", [128, SB * 128], BF, kind="ExternalInput")
    ve2 = nc.dram_tensor("ve2", [128, SB * 64], BF, kind="ExternalInput")
    wo2 = nc.dram_tensor("wo2", [2, 128, E], BF, kind="ExternalInput")
    mkd = nc.dram_tensor("mkd", [128, 2 * 128], BF, kind="ExternalInput")
    mkf = nc.dram_tensor("mkf", [128, 2 * 128], BF, kind="ExternalInput")
    y = nc.dram_tensor("y", [SB, 128, E], BF, kind="ExternalOutput")

    with tile.TileContext(nc) as tc, ExitStack() as top:
        const = top.enter_context(tc.tile_pool(name="const", bufs=1))
        persist = top.enter_context(tc.tile_pool(name="persist", bufs=1))

        # ---- persistent activations ----
        qkT = [persist.tile([128, 3, GW * 128], BF, name=f"qkT{g}")
               for g in range(NG)]
        aoT = [persist.tile([128, S], BF, name=f"aoT{i}") for i in range(2)]
        vex = [persist.tile([128, 128], BF, name=f"vex{s}")
               for s in range(SB)]
        for s in range(SB):
            nc.gpsimd.memset(vex[s][:, 64:128], 1.0)
        # rstd per group: [:, :, 0:HPC] = q heads, [:, :, HPC] = k
        rg = [persist.tile([128, GW, 5], F32, name=f"rg{g}")
              for g in range(NG)]
        gate_sb = persist.tile([128, SB], F32)
        ge = persist.tile([128, SB], F32)
        gd = persist.tile([128, SB], F32)
        sig = persist.tile([128, SB], F32)

        maskd_sb = const.tile([128, 2, 128], BF)
        maskf_sb = const.tile([128, 2, 128], BF)
        cs_sb = const.tile([128, SB, 128], BF)
        ve_sb = const.tile([128, SB, 64], BF)
        wo_sb = [const.tile([128, E], BF, name=f"wo{i}") for i in range(2)]
        # weights in 2+2+4 k8-chunks: HWDGE costs ~625ns per DMA regardless
        # of size, so keep the count low, but quarter the first chunks so
        # the opening projection can issue ~1.5us sooner
        WQP = [(0, 2), (2, 2), (4, 4)]
        wq_h = [const.tile([128, n, QKW], BF, name=f"wqh{i}")
                for i, (k0, n) in enumerate(WQP)]

        with ExitStack() as p1:
            xpool = p1.enter_context(tc.tile_pool(name="xg", bufs=1))
            work = p1.enter_context(tc.tile_pool(name="work", bufs=1))
            big_psp = p1.enter_context(tc.tile_pool(name="big_ps", bufs=1,
                                                    space="PSUM"))
            strip_psp = p1.enter_context(tc.tile_pool(name="strip", bufs=1,
                                                      space="PSUM"))
            acc_psp = p1.enter_context(tc.tile_pool(name="acc", bufs=1,
                                                    space="PSUM"))
            expp = p1.enter_context(tc.tile_pool(name="expp", bufs=1))
            ep = p1.enter_context(tc.tile_pool(name="ep", bufs=1))
            yp = p1.enter_context(tc.tile_pool(name="yp", bufs=1))

            xg = {}
            st = {}

            # group-0 x chunks interleaved with the weight chunks on two
            # HWDGE queues
            xg0_h = [xpool.tile([128, n, 256], BF, tag=f"xg0{i}",
                                name=f"xg0h{i}", bufs=1)
                     for i, (k0, n) in enumerate(WQP)]
            for i, (k0, n) in enumerate(WQP):
                nc.sync.dma_start(
                    wq_h[i], wqg[:, k0 * QKW:(k0 + n) * QKW]
                    .rearrange("p (k c) -> p k c", k=n))
                nc.scalar.dma_start(
                    xg0_h[i], xg2[0, :, k0 * 256:(k0 + n) * 256]
                    .rearrange("p (k c) -> p k c", k=n))

            def wq_part(k8):
                i = 0 if k8 < 2 else (1 if k8 < 4 else 2)
                return wq_h[i][:, k8 - WQP[i][0], :]

            def xg0_part(k8):
                i = 0 if k8 < 2 else (1 if k8 < 4 else 2)
                return xg0_h[i][:, k8 - WQP[i][0], :]

            def load_group(g):
                t = xpool.tile([128, 8, 256], BF, tag="xg", name="xg_t", bufs=5)
                nc.sync.dma_start(
                    t, xg2[g, :, :].rearrange("p (k c) -> p k c", k=8))
                xg[g] = t

            load_group(1)
            # remaining constants on the Act HWDGE queue, behind nothing hot
            nc.scalar.dma_start(cs_sb,
                                cs2[:, :].rearrange("p (s c) -> p s c", s=SB))
            nc.scalar.dma_start(ve_sb,
                                ve2[:, :].rearrange("p (s c) -> p s c", s=SB))
            nc.scalar.dma_start(maskd_sb,
                                mkd[:, :].rearrange("p (h x) -> p h x", h=2))
            nc.scalar.dma_start(maskf_sb,
                                mkf[:, :].rearrange("p (h x) -> p h x", h=2))
            for i in range(2):
                nc.scalar.dma_start(wo_sb[i], wo2[i, :, :])

            def stage_a(g):
                # rotate the projection outputs through ALL THREE psum tags
                # (their slots are the same 2KB size; strips/acc are idle
                # until the attention loop) so the projections aren't
                # throttled by the 2-deep big-tag <-> stage_b round-trip
                r = g % 3
                if r == 1:
                    stp = strip_psp.tile([128, 2, 512], F32, tag="strip",
                                         name="strip", bufs=2)
                    ps_pair = [stp[:, li, 0:QKW] for li in range(GW)]
                elif r == 2:
                    ps_pair = []
                    for li in range(GW):
                        at = acc_psp.tile([128, 2, 256], F32, tag="acc",
                                          name="acc", bufs=2)
                        ps_pair.append(
                            at[:].rearrange("p a b -> p (a b)")[:, 0:QKW])
                else:
                    ps_pair = []
                    for li in range(GW):
                        big = big_psp.tile([128, 512], F32, tag="big",
                                           name="big_ps", bufs=2)
                        ps_pair.append(big[:, 0:QKW])
                for li in range(GW):
                    lcol = slice(li * 128, (li + 1) * 128)
                    for k8 in range(8):
                        xs = (xg0_part(k8)[:, lcol] if g == 0
                              else xg[g][:, k8, lcol])
                        nc.tensor.matmul(ps_pair[li], xs, wq_part(k8),
                                         start=(k8 == 0), stop=(k8 == 7),
                                         skip_group_check=True)
                st[g] = dict(ps=ps_pair)

            def stage_b(g):
                # one wide PSUM->SBUF copy per s-block (frees the big PSUM
                # buf sooner for the next projection); v/gate extracted from
                # SBUF on Pool, off the critical chain
                s_ = st[g]
                qkvsb = work.tile([128, GW, QKW], BF, tag="qkvsb",
                                  name="qkvsb", bufs=3)
                for li in range(GW):
                    ps_t = s_["ps"][li]
                    nc.scalar.copy(qkvsb[:, li, :], ps_t)
                for li in range(GW):
                    sb = g * GW + li
                    nc.gpsimd.tensor_copy(vex[sb][:, 0:64],
                                          qkvsb[:, li, RW:RW + 64])
                    nc.gpsimd.tensor_copy(gate_sb[:, sb:sb + 1],
                                          qkvsb[:, li, RW + 64:QKW])
                s_["qkvsb"] = qkvsb
                del s_["ps"]

            def csbc(g, off, width):
                # [128, li(2), 5-head bcast, width] view of cos/sin table
                return bass.AP(tensor=cs_sb.tensor,
                               offset=cs_sb.offset + (g * GW) * 128 + off,
                               ap=[list(cs_sb.ap[0]), [128, GW], [0, 5],
                                   [1, width]])

            def stage_c(g):
                s_ = st[g]
                qsb = s_["qkvsb"]
                qk5 = bass.AP(tensor=qsb.tensor, offset=qsb.offset,
                              ap=[list(qsb.ap[0]), [QKW, GW], [D, 5], [1, D]])
                tm1 = work.tile([128, GW, RW], BF, tag="tm1", name="tm1",
                                bufs=3)
                tm2 = work.tile([128, GW, RW], BF, tag="tm2", name="tm2",
                                bufs=3)
                tm1v = tm1[:].rearrange("p l (h d) -> p l h d", h=5)
                tm2v = tm2[:].rearrange("p l (h d) -> p l h d", h=5)
                nc.vector.tensor_mul(tm1v, qk5, csbc(g, 0, D))
                nc.gpsimd.tensor_mul(tm2v[:, :, :, 0:HALF],
                                     qk5[:, :, :, HALF:D], csbc(g, D, HALF))
                nc.vector.tensor_mul(tm2v[:, :, :, HALF:D],
                                     qk5[:, :, :, 0:HALF],
                                     csbc(g, D + HALF, HALF))
                qk_r = work.tile([128, GW, RW], BF, tag="qkr", name="qk_r",
                                 bufs=4)
                nc.vector.tensor_add(qk_r, tm1, tm2)
                s_["qk_r"] = qk_r

            def stage_d(g):
                s_ = st[g]
                qk_r = s_["qk_r"]
                sq = work.tile([128, GW, RW], BF, tag="sq", name="sq", bufs=3)
                nc.vector.tensor_mul(sq, qk_r, qk_r)
                ssum = work.tile([128, GW, 5], BF, tag="ssum", name="ssum",
                                 bufs=3)
                with nc.allow_low_precision(reason="rmsnorm ssum bf16: "
                                            "0.4% on ssum -> 0.2% on rstd"):
                    nc.vector.reduce_sum(
                        ssum[:].rearrange("p a b -> p (a b)"),
                        sq[:].rearrange("p l (h d) -> p (l h) d", h=5),
                        axis=mybir.AxisListType.X)
                # rstd_q = exp(-0.5*ln(ssum)); rstd_k = exp(-0.5*ln(ssum/64))
                # = sqrt(64/ssum) -- the D-scale enters via the Ln input
                # scale (rmsnorm is scale-invariant, so it can't come from a
                # host-side Wk scale)
                lt = work.tile([128, GW, 5], F32, tag="lt", name="lt", bufs=3)
                nc.scalar.activation(lt[:, :, 0:HPC], ssum[:, :, 0:HPC],
                                     AF.Ln, bias=0.0, scale=1.0)
                nc.scalar.activation(lt[:, :, HPC:5], ssum[:, :, HPC:5],
                                     AF.Ln, bias=0.0, scale=1.0 / D)
                nc.scalar.activation(rg[g][:], lt, AF.Exp, bias=0.0,
                                     scale=-0.5)

            def stage_e(g):
                s_ = st[g]
                qn = work.tile([128, GW, QNW], BF, tag="qn", name="qn", bufs=4)
                rbc = bass.AP(tensor=rg[g].tensor, offset=rg[g].offset,
                              ap=[list(rg[g].ap[0]), [5, GW], [1, HPC],
                                  [0, D]])
                nc.vector.tensor_mul(
                    qn[:, :, 0:HD].rearrange("p l (h d) -> p l h d", h=HPC),
                    s_["qk_r"][:, :, 0:HD].rearrange("p l (h d) -> p l h d",
                                                     h=HPC),
                    rbc)
                # roped k (unnormalized, pre-scaled 1/8), duplicated twice
                kin = bass.AP(tensor=s_["qk_r"].tensor,
                              offset=s_["qk_r"].offset + HD,
                              ap=[list(s_["qk_r"].ap[0]), [RW, GW], [0, 2],
                                  [1, D]])
                kout = bass.AP(tensor=qn.tensor, offset=qn.offset + HD,
                               ap=[list(qn.ap[0]), [QNW, GW], [D, 2], [1, D]])
                nc.vector.tensor_copy(kout, kin)
                s_["qn"] = qn

            def stage_f(g):
                s_ = st.pop(g)
                qn = s_["qn"]
                for li in range(GW):
                    lcol = slice(li * 128, (li + 1) * 128)
                    nc.sync.dma_start(qkT[g][:, :, lcol], qn[:, li, :],
                                      transpose=True)

            def gate_group(g):
                # per-group gate: spreads the sigmoid + vex gating over the
                # phase-1 ticks instead of a 16-op DVE burst at t=NG
                gcol = slice(g * GW, (g + 1) * GW)
                nc.scalar.activation(ge[:, gcol], gate_sb[:, gcol], AF.Exp,
                                     bias=0.0, scale=-1.0)
                nc.vector.tensor_scalar_add(gd[:, gcol], ge[:, gcol], 1.0)
                nc.vector.reciprocal(sig[:, gcol], gd[:, gcol])
                for sb in range(g * GW, (g + 1) * GW):
                    nc.vector.scalar_tensor_tensor(
                        out=vex[sb][:, 0:64], in0=ve_sb[:, sb, :],
                        scalar=sig[:, sb:sb + 1], in1=vex[sb][:, 0:64],
                        op0=ALU.mult, op1=ALU.add)

            # ---------------- attention j-step pieces ----------------
            exps = {0: {}, 1: {}}
            acst = {}

            def mm1(ps, j):
                nq = min(j + WB + 1, SB) - j
                et = expp.tile([128, 2, (WB + 1) * 128], BF, tag=f"exp{ps}",
                               name=f"exp{ps}", bufs=10)
                exps[ps][j] = et
                kt = qkT[j // 2]
                jcol = slice((j % 2) * 128, (j % 2) * 128 + 128)
                rk_ap = rg[j // 2][:, (j % 2), HPC:5]
                # group-aligned segments: [1] if j odd, then pairs, tail [1]
                segs = []
                b = j
                if b % 2 == 1:
                    segs.append((b, 1))
                    b += 1
                while b + 1 < j + nq:
                    segs.append((b, 2))
                    b += 2
                if b < j + nq:
                    segs.append((b, 1))
                # pack segments into strip buffers of <= 4 blocks, exp per buf
                si = 0
                off = 0
                while si < len(segs):
                    take = []
                    blk = 0
                    while si < len(segs) and blk + segs[si][1] <= 4:
                        take.append(segs[si])
                        blk += segs[si][1]
                        si += 1
                    stp = strip_psp.tile([128, 2, 512], F32, tag="strip",
                                         name="strip", bufs=2)
                    co = 0
                    for b0, nb in take:
                        w = nb * 128
                        g0 = b0 // 2
                        qcol = slice((b0 % 2) * 128, (b0 % 2) * 128 + w)
                        nc.tensor.matmul(stp[:, 0, co:co + w],
                                         kt[0:64, 2, jcol],
                                         qkT[g0][0:64, ps, qcol],
                                         start=True, stop=True,
                                         skip_group_check=True)
                        nc.tensor.matmul(stp[:, 1, co:co + w],
                                         kt[64:128, 2, jcol],
                                         qkT[g0][64:128, ps, qcol],
                                         start=True, stop=True,
                                         skip_group_check=True)
                        co += w
                    cw = blk * 128
                    nc.scalar.activation(et[:, :, off:off + cw],
                                         stp[:, :, 0:cw], AF.Exp,
                                         bias=0.0, scale=rk_ap)
                    if off == 0:
                        nc.vector.tensor_mul(et[:, :, 0:128], et[:, :, 0:128],
                                             maskd_sb)
                    off += cw
                if nq == WB + 1:
                    fcol = slice(WB * 128, (WB + 1) * 128)
                    nc.vector.tensor_mul(et[:, :, fcol], et[:, :, fcol],
                                         maskf_sb)

            def mm2pair(ps, m):
                q0, q1 = 2 * m, 2 * m + 1
                a = acc_psp.tile([128, 2, 256], F32, tag="acc", name="acc",
                                 bufs=2)
                first = True
                if q0 - WB >= 0:
                    jj = q0 - WB
                    o0 = (q0 - jj) * 128
                    nc.tensor.matmul(
                        a[:, :, 0:128], vex[jj][:],
                        exps[ps][jj][:, :, o0:o0 + 128],
                        start=True, stop=False, skip_group_check=True)
                    first = False
                for jj in range(max(0, q1 - WB), q0 + 1):
                    off = (q0 - jj) * 128
                    nc.tensor.matmul(a, vex[jj][:],
                                     exps[ps][jj][:, :, off:off + 256],
                                     start=first, stop=False,
                                     skip_group_check=True)
                    first = False
                nc.tensor.matmul(a[:, :, 128:256], vex[q1][:],
                                 exps[ps][q1][:, :, 0:128],
                                 start=False, stop=True,
                                 skip_group_check=True)
                acst[(ps, m)] = dict(a=a)

            def epi1(ps, m):
                s_ = acst[(ps, m)]
                rec = ep.tile([64, 2, 256], BF, tag=f"rec{ps}",
                              name=f"rec{ps}", bufs=2)
                with nc.allow_low_precision(reason="softmax denom recip"):
                    nc.vector.reciprocal(rec, s_["a"][64:128, :, :])
                s_["rec"] = rec

            def epi2(ps, m):
                s_ = acst.pop((ps, m))
                a, rec = s_["a"], s_["rec"]
                scol = slice(2 * m * 128, (2 * m + 2) * 128)
                # write aoT halves straight from PSUM x rec (PSUM in0 lets
                # the out partition base differ from the input bases)
                nc.vector.tensor_mul(aoT[ps][0:64, scol], a[0:64, 0, :],
                                     rec[:, 0, :])
                nc.vector.tensor_mul(aoT[ps][64:128, scol], a[0:64, 1, :],
                                     rec[:, 1, :])

            def mm3(sb, y_t, half, tail=False):
                scol = slice(sb * 128, (sb + 1) * 128)
                for nch in range(2):
                    y_ps = big_psp.tile([128, 512], F32, tag="big",
                                        name="y_ps", bufs=2)
                    for i in range(2):
                        nc.tensor.matmul(y_ps, aoT[i][:, scol],
                                         wo_sb[i][:, nch * 512:(nch + 1) * 512],
                                         start=(i == 0), stop=(i == 1),
                                         skip_group_check=True)
                    # tail: Act is idle after the last exp, so split the
                    # final copies across both engines to shorten the drain
                    eng = nc.scalar.copy if (tail and nch == 1) else \
                        nc.vector.tensor_copy
                    eng(y_t[:, half, nch * 512:(nch + 1) * 512], y_ps)

            yts = {}

            def jblock_mm1(j):
                if j < SB:
                    mm1(0, j)
                    mm1(1, j)

            def jblock(j):
                # mm3 for the pair m_ is split across this odd tick and the
                # following even one: 2 y_ps PSUM allocs per tick instead of
                # 4 keeps the DVE y-copy off mm3's big-tag rotation wait
                if j % 2 == 1:
                    if j >= 3 and (j - 3) // 2 < WB - 1:
                        m_ = (j - 3) // 2
                        for ps in range(2):
                            epi2(ps, m_)
                        y_t = yp.tile([128, 2, E], BF, tag="ysb", name="y_t",
                                      bufs=2)
                        yts[m_] = y_t
                        mm3(2 * m_, y_t, 0, tail=(m_ >= 5))
                    if (j - 1) // 2 < WB:
                        m = (j - 1) // 2
                        mm2pair(0, m)
                        mm2pair(1, m)
                        if m == WB - 1:
                            # final pair: recip right behind the last acc so
                            # the wind-down finishes a tick earlier
                            epi1(0, m)
                            epi1(1, m)
                else:
                    if j >= 4 and (j - 4) // 2 < WB - 1:
                        m_ = (j - 4) // 2
                        y_t = yts.pop(m_)
                        mm3(2 * m_ + 1, y_t, 1, tail=(m_ >= 3))
                        nc.sync.dma_start(
                            y[2 * m_:2 * m_ + 2, :, :]
                            .rearrange("s p e -> p s e"), y_t)
                    if j >= 2 and j // 2 - 1 < WB - 1:
                        for ps in range(2):
                            epi1(ps, j // 2 - 1)
                    if j == 2 * WB:
                        # final pair: epi2 + both mm3 halves + split DMAs in
                        # this tick instead of spilling into j=17
                        m_ = WB - 1
                        for ps in range(2):
                            epi2(ps, m_)
                        y_t = yp.tile([128, 2, E], BF, tag="ysb", name="y_t",
                                      bufs=2)
                        mm3(2 * m_, y_t, 0, tail=True)
                        nc.sync.dma_start(
                            y[2 * m_:2 * m_ + 1, :, :]
                            .rearrange("s p e -> p s e"), y_t[:, 0:1, :])
                        mm3(2 * m_ + 1, y_t, 1, tail=True)
                        nc.sync.dma_start(
                            y[2 * m_ + 1:2 * m_ + 2, :, :]
                            .rearrange("s p e -> p s e"), y_t[:, 1:2, :])

            # ---------------- merged tick loop ----------------
            for t in range(18):
                if t + 2 < NG:
                    load_group(t + 2)
                if 0 <= t - 3 < NG:
                    stage_f(t - 3)
                if 0 <= t - 1 < NG:
                    stage_b(t - 1)
                if 0 <= t - 2 < NG:
                    stage_d(t - 2)
                    stage_e(t - 2)
                if 0 <= t - 1 < NG:
                    stage_c(t - 1)
                if 0 <= t - 3 < NG:
                    gate_group(t - 3)
                if t < NG:
                    stage_a(t)
                # both js' score strips first: Act's exp queue stays a
                # half-tick ahead of the mm2 reads of the fresh et tiles
                for j in (2 * (t - 7) - 1, 2 * (t - 7)):
                    if 0 <= j < SB + 2:
                        jblock_mm1(j)
                for j in (2 * (t - 7) - 1, 2 * (t - 7)):
                    if 0 <= j < SB + 1:
                        jblock(j)

    nc.compile()
    return nc


def _prep_core_inputs(c, x, ve, cos, sin, Wq, Wk, Wv, Wo, Wg):
    b = c // TP
    h0 = (c % TP) * HD
    xT = np.ascontiguousarray(x[b].T).astype(BF_NP)          # [E, S]
    xg2 = np.empty((NG, 128, 8 * 256), BF_NP)
    for g in range(NG):
        for k8 in range(8):
            xg2[g, :, k8 * 256:(k8 + 1) * 256] = \
                xT[k8 * 128:(k8 + 1) * 128, g * 256:(g + 1) * 256]
    wg_pad = np.zeros((E, 1), np.float32)
    wg_pad[:GATE_CH, 0] = Wg[:, 0]
    wqkv = np.concatenate([Wq[:, h0:h0 + HD], Wk, Wv, wg_pad], axis=1)
    wqg = np.ascontiguousarray(
        wqkv.reshape(8, 128, QKW).transpose(1, 0, 2)
        .reshape(128, 8 * QKW)).astype(BF_NP)
    ccss = np.concatenate([cos, cos, sin, -sin], axis=1)     # [S, 128]
    cs2 = np.ascontiguousarray(
        ccss.reshape(SB, 128, 128).transpose(1, 0, 2).reshape(128, SB * 128)
    ).astype(BF_NP)
    ve2 = np.ascontiguousarray(
        (2.0 * ve[b]).reshape(SB, 128, 64).transpose(1, 0, 2)
        .reshape(128, SB * 64)).astype(BF_NP)
    wo2 = np.ascontiguousarray(
        Wo[h0:h0 + HD, :].reshape(2, 128, E)).astype(BF_NP)
    ii = np.arange(128)
    md = (ii[None, :] >= ii[:, None]).astype(np.float32)     # [ki, qi]
    mf = 1.0 - md
    mkd = np.tile(md, (1, 2)).reshape(128, 256).astype(BF_NP)
    mkf = np.tile(mf, (1, 2)).reshape(128, 256).astype(BF_NP)
    return dict(xg2=xg2, wqg=wqg, cs2=cs2, ve2=ve2, wo2=wo2,
                mkd=mkd, mkf=mkf)


def kernel(x, ve, cos, sin, Wq, Wk, Wv, Wo, Wg, window_size):
    assert int(window_size) == WIN
    x = np.asarray(x, np.float32)
    ve = np.asarray(ve, np.float32)
    cos = np.asarray(cos, np.float32)
    sin = np.asarray(sin, np.float32)
    Wq = np.asarray(Wq, np.float32)
    Wk = np.asarray(Wk, np.float32)
    Wv = np.asarray(Wv, np.float32)
    Wo = np.asarray(Wo, np.float32)
    Wg = np.asarray(Wg, np.float32)

    if _NC_CACHE[0] is None:
        _NC_CACHE[0] = _build()
    nc = _NC_CACHE[0]

    in_maps = [_prep_core_inputs(c, x, ve, cos, sin, Wq, Wk, Wv, Wo, Wg)
               for c in range(NCORES)]
    res = run_bass_kernel_spmd(nc, in_maps, core_ids=list(range(NCORES)),
                               trace=TRACE)
    LAST_RESULT[0] = res

    out = np.zeros((B, S, E), np.float32)
    for c in range(NCORES):
        out[c // TP] += res.results[c]["y"].astype(np.float32).reshape(S, E)
    return out


# revision 65
# speedup vs baseline: 1.0114x; 1.0097x over previous
"""Trainium2 Bass kernel for MQA sliding-window causal self-attention.

Sharding: 8 cores = DP(batch=2) x TP(head-groups=4). Each core computes 4 of
16 query heads for one batch element, shared KV head replicated. Host
pre-packs transposed/padded bf16 layouts (Wk pre-scaled by 1/8 so the
softmax scale folds into k's rmsnorm term); gathers + sums the 4 TP partial
outputs per batch element.

v2 vs baseline (152349ns):
  - ONE act table for the whole kernel: rstd = exp(-0.5*ln(ssum)) via the
    natural_log_exp_and_others set (act-table monkeypatch strips ln/exp from
    other sets so the greedy CFG pass can't thrash Sqrt<->Exp tables: was 11
    LoadActFuncSet = 14.1us on Act + PE stalls).
  - startup: wq + first x group split into 8 per-k8 DMA slices on two HWDGE
    queues so the first projection matmul issues at ~0.6us (was 7.9us gap).
  - stage_b PSUM->SBUF copies + y output copies moved Act->Pool (gpsimd);
    square moved Act->DVE (bf16 2x self-mul); Act runs ~only exp.
  - edge masks gpsimd->DVE (bf16 2x) and merged across the 2 head-pair sets
    (one et tile [128,2ps,2,1152] per j-block).
  - epilogue: softmax divide writes aoT halves directly (PSUM in0 allows
    out-partition-base offset), killing the aop intermediate, its copies,
    and the odd-half SBUF DMA.
"""
import numpy as np
import ml_dtypes
from contextlib import ExitStack

import concourse.bass as bass
import concourse.tile as tile
import concourse.mybir as mybir
from concourse import bacc
from concourse.bass_utils import run_bass_kernel_spmd

# ---- act-table selection patch: keep ln/exp ONLY in the one set that has
# both, so the greedy table-load pass emits a single LoadActFuncSet ----
import concourse.hw_specs as _hs
import concourse.bacc as _bacc_mod

_ONLY_SET = "natural_log_exp_and_others"
_orig_get_tables = _hs.get_activation_tables


def _patched_tables(arch):
    tabs = dict(_orig_get_tables(arch))
    keep = tabs[_ONLY_SET]
    return {k: (v if k == _ONLY_SET else (v - keep)) for k, v in tabs.items()}


_bacc_mod.get_activation_tables = _patched_tables

F32 = mybir.dt.float32
BF = mybir.dt.bfloat16
AF = mybir.ActivationFunctionType
ALU = mybir.AluOpType
BF_NP = ml_dtypes.bfloat16

B, S, E, H, KV, D = 2, 2048, 1024, 16, 1, 64
HALF = D // 2
GATE_CH = 32
WIN = 1024
NCORES = 8
TP = 4
HPC = H // TP            # heads per core = 4
HD = HPC * D             # per-core q width = 256
SB = S // 128            # 16 s-blocks
WB = WIN // 128          # 8 window blocks
QKW = HD + 2 * D + 1     # 385 (q 256 | k 64 | v 64 | gate 1)
RW = HD + D              # 320 roped width (4 q heads + k)
QNW = HD + 2 * D         # 384 qn width (q 256 | k | k-dup)
GW = 2                   # s-blocks per phase-1 group
NG = SB // GW            # 8 groups

TRACE = False
LAST_RESULT = [None]
_NC_CACHE = [None]


def _build():
    nc = bacc.Bacc()

    xg2 = nc.dram_tensor("xg2", [NG, 128, 8 * 256], BF, kind="ExternalInput")
    wqg = nc.dram_tensor("wqg", [128, 8 * QKW], BF, kind="ExternalInput")
    cs2 = nc.dram_tensor("cs2", [128, SB * 128], BF, kind="ExternalInput")
    ve2 = nc.dram_tensor("ve2", [128, SB * 64], BF, kind="ExternalInput")
    wo2 = nc.dram_tensor("wo2", [2, 128, E], BF, kind="ExternalInput")
    mkd = nc.dram_tensor("mkd", [128, 2 * 128], BF, kind="ExternalInput")
    mkf = nc.dram_tensor("mkf", [128, 2 * 128], BF, kind="ExternalInput")
    y = nc.dram_tensor("y", [SB, 128, E], BF, kind="ExternalOutput")

    with tile.TileContext(nc) as tc, ExitStack() as top:
        const = top.enter_context(tc.tile_pool(name="const", bufs=1))
        persist = top.enter_context(tc.tile_pool(name="persist", bufs=1))

        # ---- persistent activations ----
        qkT = [persist.tile([128, 3, GW * 128], BF, name=f"qkT{g}")
               for g in range(NG)]
        aoT = [persist.tile([128, S], BF, name=f"aoT{i}") for i in range(2)]
        vex = [persist.tile([128, 128], BF, name=f"vex{s}")
               for s in range(SB)]
        for s in range(SB):
            nc.gpsimd.memset(vex[s][:, 64:128], 1.0)
        # rstd per group: [:, :, 0:HPC] = q heads, [:, :, HPC] = k
        rg = [persist.tile([128, GW, 5], F32, name=f"rg{g}")
              for g in range(NG)]
        gate_sb = persist.tile([128, SB], F32)
        ge = persist.tile([128, SB], F32)
        gd = persist.tile([128, SB], F32)
        sig = persist.tile([128, SB], F32)

        maskd_sb = const.tile([128, 2, 128], BF)
        maskf_sb = const.tile([128, 2, 128], BF)
        cs_sb = const.tile([128, SB, 128], BF)
        ve_sb = const.tile([128, SB, 64], BF)
        wo_sb = [const.tile([128, E], BF, name=f"wo{i}") for i in range(2)]
        # weights in 2+2+4 k8-chunks: HWDGE costs ~625ns per DMA regardless
        # of size, so keep the count low, but quarter the first chunks so
        # the opening projection can issue ~1.5us sooner
        WQP = [(0, 2), (2, 2), (4, 4)]
        wq_h = [const.tile([128, n, QKW], BF, name=f"wqh{i}")
                for i, (k0, n) in enumerate(WQP)]

        with ExitStack() as p1:
            xpool = p1.enter_context(tc.tile_pool(name="xg", bufs=1))
            work = p1.enter_context(tc.tile_pool(name="work", bufs=1))
            big_psp = p1.enter_context(tc.tile_pool(name="big_ps", bufs=1,
                                                    space="PSUM"))
            strip_psp = p1.enter_context(tc.tile_pool(name="strip", bufs=1,
                                                      space="PSUM"))
            acc_psp = p1.enter_context(tc.tile_pool(name="acc", bufs=1,
                                                    space="PSUM"))
            expp = p1.enter_context(tc.tile_pool(name="expp", bufs=1))
            ep = p1.enter_context(tc.tile_pool(name="ep", bufs=1))
            yp = p1.enter_context(tc.tile_pool(name="yp", bufs=1))

            xg = {}
            st = {}

            # group-0 x chunks interleaved with the weight chunks on two
            # HWDGE queues
            xg0_h = [xpool.tile([128, n, 256], BF, tag=f"xg0{i}",
                                name=f"xg0h{i}", bufs=1)
                     for i, (k0, n) in enumerate(WQP)]
            for i, (k0, n) in enumerate(WQP):
                nc.sync.dma_start(
                    wq_h[i], wqg[:, k0 * QKW:(k0 + n) * QKW]
                    .rearrange("p (k c) -> p k c", k=n))
                nc.scalar.dma_start(
                    xg0_h[i], xg2[0, :, k0 * 256:(k0 + n) * 256]
                    .rearrange("p (k c) -> p k c", k=n))

            def wq_part(k8):
                i = 0 if k8 < 2 else (1 if k8 < 4 else 2)
                return wq_h[i][:, k8 - WQP[i][0], :]

            def xg0_part(k8):
                i = 0 if k8 < 2 else (1 if k8 < 4 else 2)
                return xg0_h[i][:, k8 - WQP[i][0], :]

            def load_group(g):
                t = xpool.tile([128, 8, 256], BF, tag="xg", name="xg_t", bufs=5)
                nc.sync.dma_start(
                    t, xg2[g, :, :].rearrange("p (k c) -> p k c", k=8))
                xg[g] = t

            load_group(1)
            # remaining constants on the Act HWDGE queue, behind nothing hot
            nc.scalar.dma_start(cs_sb,
                                cs2[:, :].rearrange("p (s c) -> p s c", s=SB))
            nc.scalar.dma_start(ve_sb,
                                ve2[:, :].rearrange("p (s c) -> p s c", s=SB))
            nc.scalar.dma_start(maskd_sb,
                                mkd[:, :].rearrange("p (h x) -> p h x", h=2))
            nc.scalar.dma_start(maskf_sb,
                                mkf[:, :].rearrange("p (h x) -> p h x", h=2))
            for i in range(2):
                nc.scalar.dma_start(wo_sb[i], wo2[i, :, :])

            def stage_a(g):
                # rotate the projection outputs through ALL THREE psum tags
                # (their slots are the same 2KB size; strips/acc are idle
                # until the attention loop) so the projections aren't
                # throttled by the 2-deep big-tag <-> stage_b round-trip
                r = g % 3
                if r == 1:
                    stp = strip_psp.tile([128, 2, 512], F32, tag="strip",
                                         name="strip", bufs=2)
                    ps_pair = [stp[:, li, 0:QKW] for li in range(GW)]
                elif r == 2:
                    ps_pair = []
                    for li in range(GW):
                        at = acc_psp.tile([128, 2, 256], F32, tag="acc",
                                          name="acc", bufs=2)
                        ps_pair.append(
                            at[:].rearrange("p a b -> p (a b)")[:, 0:QKW])
                else:
                    ps_pair = []
                    for li in range(GW):
                        big = big_psp.tile([128, 512], F32, tag="big",
                                           name="big_ps", bufs=2)
                        ps_pair.append(big[:, 0:QKW])
                for li in range(GW):
                    lcol = slice(li * 128, (li + 1) * 128)
                    for k8 in range(8):
                        xs = (xg0_part(k8)[:, lcol] if g == 0
                              else xg[g][:, k8, lcol])
                        nc.tensor.matmul(ps_pair[li], xs, wq_part(k8),
                                         start=(k8 == 0), stop=(k8 == 7),
                                         skip_group_check=True)
                st[g] = dict(ps=ps_pair)

            def stage_b(g):
                # one wide PSUM->SBUF copy per s-block (frees the big PSUM
                # buf sooner for the next projection); v/gate extracted from
                # SBUF on Pool, off the critical chain
                s_ = st[g]
                qkvsb = work.tile([128, GW, QKW], BF, tag="qkvsb",
                                  name="qkvsb", bufs=3)
                for li in range(GW):
                    ps_t = s_["ps"][li]
                    nc.scalar.copy(qkvsb[:, li, :], ps_t)
                for li in range(GW):
                    sb = g * GW + li
                    nc.gpsimd.tensor_copy(vex[sb][:, 0:64],
                                          qkvsb[:, li, RW:RW + 64])
                    nc.gpsimd.tensor_copy(gate_sb[:, sb:sb + 1],
                                          qkvsb[:, li, RW + 64:QKW])
                s_["qkvsb"] = qkvsb
                del s_["ps"]

            def csbc(g, off, width):
                # [128, li(2), 5-head bcast, width] view of cos/sin table
                return bass.AP(tensor=cs_sb.tensor,
                               offset=cs_sb.offset + (g * GW) * 128 + off,
                               ap=[list(cs_sb.ap[0]), [128, GW], [0, 5],
                                   [1, width]])

            def stage_c(g):
                s_ = st[g]
                qsb = s_["qkvsb"]
                qk5 = bass.AP(tensor=qsb.tensor, offset=qsb.offset,
                              ap=[list(qsb.ap[0]), [QKW, GW], [D, 5], [1, D]])
                tm1 = work.tile([128, GW, RW], BF, tag="tm1", name="tm1",
                                bufs=3)
                tm2 = work.tile([128, GW, RW], BF, tag="tm2", name="tm2",
                                bufs=3)
                tm1v = tm1[:].rearrange("p l (h d) -> p l h d", h=5)
                tm2v = tm2[:].rearrange("p l (h d) -> p l h d", h=5)
                nc.vector.tensor_mul(tm1v, qk5, csbc(g, 0, D))
                nc.gpsimd.tensor_mul(tm2v[:, :, :, 0:HALF],
                                     qk5[:, :, :, HALF:D], csbc(g, D, HALF))
                nc.vector.tensor_mul(tm2v[:, :, :, HALF:D],
                                     qk5[:, :, :, 0:HALF],
                                     csbc(g, D + HALF, HALF))
                qk_r = work.tile([128, GW, RW], BF, tag="qkr", name="qk_r",
                                 bufs=4)
                nc.vector.tensor_add(qk_r, tm1, tm2)
                s_["qk_r"] = qk_r

            def stage_d(g):
                s_ = st[g]
                qk_r = s_["qk_r"]
                sq = work.tile([128, GW, RW], BF, tag="sq", name="sq", bufs=3)
                # Act while the j-loop start hangs on this chain, DVE once
                # the exp stream owns Act
                if g <= 4:
                    nc.scalar.square(sq, qk_r)
                else:
                    nc.vector.tensor_mul(sq, qk_r, qk_r)
                ssum = work.tile([128, GW, 5], BF, tag="ssum", name="ssum",
                                 bufs=3)
                with nc.allow_low_precision(reason="rmsnorm ssum bf16: "
                                            "0.4% on ssum -> 0.2% on rstd"):
                    nc.vector.reduce_sum(
                        ssum[:].rearrange("p a b -> p (a b)"),
                        sq[:].rearrange("p l (h d) -> p (l h) d", h=5),
                        axis=mybir.AxisListType.X)
                # rstd_q = exp(-0.5*ln(ssum)); rstd_k = exp(-0.5*ln(ssum/64))
                # = sqrt(64/ssum) -- the D-scale enters via the Ln input
                # scale (rmsnorm is scale-invariant, so it can't come from a
                # host-side Wk scale)
                lt = work.tile([128, GW, 5], F32, tag="lt", name="lt", bufs=3)
                nc.scalar.activation(lt[:, :, 0:HPC], ssum[:, :, 0:HPC],
                                     AF.Ln, bias=0.0, scale=1.0)
                nc.scalar.activation(lt[:, :, HPC:5], ssum[:, :, HPC:5],
                                     AF.Ln, bias=0.0, scale=1.0 / D)
                nc.scalar.activation(rg[g][:], lt, AF.Exp, bias=0.0,
                                     scale=-0.5)

            def stage_e(g):
                s_ = st[g]
                qn = work.tile([128, GW, QNW], BF, tag="qn", name="qn", bufs=4)
                rbc = bass.AP(tensor=rg[g].tensor, offset=rg[g].offset,
                              ap=[list(rg[g].ap[0]), [5, GW], [1, HPC],
                                  [0, D]])
                nc.vector.tensor_mul(
                    qn[:, :, 0:HD].rearrange("p l (h d) -> p l h d", h=HPC),
                    s_["qk_r"][:, :, 0:HD].rearrange("p l (h d) -> p l h d",
                                                     h=HPC),
                    rbc)
                # roped k (unnormalized, pre-scaled 1/8), duplicated twice
                kin = bass.AP(tensor=s_["qk_r"].tensor,
                              offset=s_["qk_r"].offset + HD,
                              ap=[list(s_["qk_r"].ap[0]), [RW, GW], [0, 2],
                                  [1, D]])
                kout = bass.AP(tensor=qn.tensor, offset=qn.offset + HD,
                               ap=[list(qn.ap[0]), [QNW, GW], [D, 2], [1, D]])
                nc.vector.tensor_copy(kout, kin)
                s_["qn"] = qn

            def stage_f(g):
                s_ = st.pop(g)
                qn = s_["qn"]
                for li in range(GW):
                    lcol = slice(li * 128, (li + 1) * 128)
                    nc.sync.dma_start(qkT[g][:, :, lcol], qn[:, li, :],
                                      transpose=True)

            def gate_group(g):
                # per-group gate: spreads the sigmoid + vex gating over the
                # phase-1 ticks instead of a 16-op DVE burst at t=NG
                gcol = slice(g * GW, (g + 1) * GW)
                nc.scalar.activation(ge[:, gcol], gate_sb[:, gcol], AF.Exp,
                                     bias=0.0, scale=-1.0)
                nc.vector.tensor_scalar_add(gd[:, gcol], ge[:, gcol], 1.0)
                nc.vector.reciprocal(sig[:, gcol], gd[:, gcol])
                for sb in range(g * GW, (g + 1) * GW):
                    nc.vector.scalar_tensor_tensor(
                        out=vex[sb][:, 0:64], in0=ve_sb[:, sb, :],
                        scalar=sig[:, sb:sb + 1], in1=vex[sb][:, 0:64],
                        op0=ALU.mult, op1=ALU.add)

            # ---------------- attention j-step pieces ----------------
            exps = {0: {}, 1: {}}
            acst = {}

            def mm1(ps, j):
                nq = min(j + WB + 1, SB) - j
                et = expp.tile([128, 2, (WB + 1) * 128], BF, tag=f"exp{ps}",
                               name=f"exp{ps}", bufs=10)
                exps[ps][j] = et
                kt = qkT[j // 2]
                jcol = slice((j % 2) * 128, (j % 2) * 128 + 128)
                rk_ap = rg[j // 2][:, (j % 2), HPC:5]
                # group-aligned segments: [1] if j odd, then pairs, tail [1]
                segs = []
                b = j
                if b % 2 == 1:
                    segs.append((b, 1))
                    b += 1
                while b + 1 < j + nq:
                    segs.append((b, 2))
                    b += 2
                if b < j + nq:
                    segs.append((b, 1))
                # pack segments into strip buffers of <= 4 blocks, exp per buf
                si = 0
                off = 0
                while si < len(segs):
                    take = []
                    blk = 0
                    while si < len(segs) and blk + segs[si][1] <= 4:
                        take.append(segs[si])
                        blk += segs[si][1]
                        si += 1
                    stp = strip_psp.tile([128, 2, 512], F32, tag="strip",
                                         name="strip", bufs=2)
                    co = 0
                    for b0, nb in take:
                        w = nb * 128
                        g0 = b0 // 2
                        qcol = slice((b0 % 2) * 128, (b0 % 2) * 128 + w)
                        nc.tensor.matmul(stp[:, 0, co:co + w],
                                         kt[0:64, 2, jcol],
                                         qkT[g0][0:64, ps, qcol],
                                         start=True, stop=True,
                                         skip_group_check=True)
                        nc.tensor.matmul(stp[:, 1, co:co + w],
                                         kt[64:128, 2, jcol],
                                         qkT[g0][64:128, ps, qcol],
                                         start=True, stop=True,
                                         skip_group_check=True)
                        co += w
                    cw = blk * 128
                    nc.scalar.activation(et[:, :, off:off + cw],
                                         stp[:, :, 0:cw], AF.Exp,
                                         bias=0.0, scale=rk_ap)
                    if off == 0:
                        nc.vector.tensor_mul(et[:, :, 0:128], et[:, :, 0:128],
                                             maskd_sb)
                    off += cw
                if nq == WB + 1:
                    fcol = slice(WB * 128, (WB + 1) * 128)
                    nc.vector.tensor_mul(et[:, :, fcol], et[:, :, fcol],
                                         maskf_sb)

            def mm2pair(ps, m):
                q0, q1 = 2 * m, 2 * m + 1
                a = acc_psp.tile([128, 2, 256], F32, tag="acc", name="acc",
                                 bufs=2)
                first = True
                if q0 - WB >= 0:
                    jj = q0 - WB
                    o0 = (q0 - jj) * 128
                    nc.tensor.matmul(
                        a[:, :, 0:128], vex[jj][:],
                        exps[ps][jj][:, :, o0:o0 + 128],
                        start=True, stop=False, skip_group_check=True)
                    first = False
                for jj in range(max(0, q1 - WB), q0 + 1):
                    off = (q0 - jj) * 128
                    nc.tensor.matmul(a, vex[jj][:],
                                     exps[ps][jj][:, :, off:off + 256],
                                     start=first, stop=False,
                                     skip_group_check=True)
                    first = False
                nc.tensor.matmul(a[:, :, 128:256], vex[q1][:],
                                 exps[ps][q1][:, :, 0:128],
                                 start=False, stop=True,
                                 skip_group_check=True)
                acst[(ps, m)] = dict(a=a)

            def epi1(ps, m):
                s_ = acst[(ps, m)]
                rec = ep.tile([64, 2, 256], BF, tag=f"rec{ps}",
                              name=f"rec{ps}", bufs=2)
                with nc.allow_low_precision(reason="softmax denom recip"):
                    nc.vector.reciprocal(rec, s_["a"][64:128, :, :])
                s_["rec"] = rec

            def epi2(ps, m):
                s_ = acst.pop((ps, m))
                a, rec = s_["a"], s_["rec"]
                scol = slice(2 * m * 128, (2 * m + 2) * 128)
                # write aoT halves straight from PSUM x rec (PSUM in0 lets
                # the out partition base differ from the input bases)
                nc.vector.tensor_mul(aoT[ps][0:64, scol], a[0:64, 0, :],
                                     rec[:, 0, :])
                nc.vector.tensor_mul(aoT[ps][64:128, scol], a[0:64, 1, :],
                                     rec[:, 1, :])

            def mm3(sb, y_t, half, tail=False):
                scol = slice(sb * 128, (sb + 1) * 128)
                for nch in range(2):
                    y_ps = big_psp.tile([128, 512], F32, tag="big",
                                        name="y_ps", bufs=2)
                    for i in range(2):
                        nc.tensor.matmul(y_ps, aoT[i][:, scol],
                                         wo_sb[i][:, nch * 512:(nch + 1) * 512],
                                         start=(i == 0), stop=(i == 1),
                                         skip_group_check=True)
                    # tail: Act is idle after the last exp, so split the
                    # final copies across both engines to shorten the drain
                    eng = nc.scalar.copy if (tail and nch == 1) else \
                        nc.vector.tensor_copy
                    eng(y_t[:, half, nch * 512:(nch + 1) * 512], y_ps)

            yts = {}

            def jblock_mm1(j):
                if j < SB:
                    mm1(0, j)
                    mm1(1, j)

            def jblock(j):
                # mm3 for the pair m_ is split across this odd tick and the
                # following even one: 2 y_ps PSUM allocs per tick instead of
                # 4 keeps the DVE y-copy off mm3's big-tag rotation wait
                if j % 2 == 1:
                    if j >= 3 and (j - 3) // 2 < WB - 1:
                        m_ = (j - 3) // 2
                        for ps in range(2):
                            epi2(ps, m_)
                        y_t = yp.tile([128, 2, E], BF, tag="ysb", name="y_t",
                                      bufs=2)
                        yts[m_] = y_t
                        mm3(2 * m_, y_t, 0, tail=(m_ >= 5))
                    if (j - 1) // 2 < WB:
                        m = (j - 1) // 2
                        mm2pair(0, m)
                        mm2pair(1, m)
                        if m == WB - 1:
                            # final pair: recip right behind the last acc so
                            # the wind-down finishes a tick earlier
                            epi1(0, m)
                            epi1(1, m)
                else:
                    if j >= 4 and (j - 4) // 2 < WB - 1:
                        m_ = (j - 4) // 2
                        y_t = yts.pop(m_)
                        mm3(2 * m_ + 1, y_t, 1, tail=(m_ >= 3))
                        nc.sync.dma_start(
                            y[2 * m_:2 * m_ + 2, :, :]
                            .rearrange("s p e -> p s e"), y_t)
                    if j >= 2 and j // 2 - 1 < WB - 1:
                        for ps in range(2):
                            epi1(ps, j // 2 - 1)
                    if j == 2 * WB:
                        # final pair: epi2 + both mm3 halves + split DMAs in
                        # this tick instead of spilling into j=17
                        m_ = WB - 1
                        for ps in range(2):
                            epi2(ps, m_)
                        y_t = yp.tile([128, 2, E], BF, tag="ysb", name="y_t",
                                      bufs=2)
                        mm3(2 * m_, y_t, 0, tail=True)
                        nc.sync.dma_start(
                            y[2 * m_:2 * m_ + 1, :, :]
                            .rearrange("s p e -> p s e"), y_t[:, 0:1, :])
                        mm3(2 * m_ + 1, y_t, 1, tail=True)
                        nc.sync.dma_start(
                            y[2 * m_ + 1:2 * m_ + 2, :, :]
                            .rearrange("s p e -> p s e"), y_t[:, 1:2, :])

            # ---------------- merged tick loop ----------------
            for t in range(18):
                if t + 2 < NG:
                    load_group(t + 2)
                if 0 <= t - 3 < NG:
                    stage_f(t - 3)
                if 0 <= t - 1 < NG:
                    stage_b(t - 1)
                if 0 <= t - 2 < NG:
                    stage_d(t - 2)
                    stage_e(t - 2)
                if 0 <= t - 1 < NG:
                    stage_c(t - 1)
                if 0 <= t - 3 < NG:
                    gate_group(t - 3)
                if t < NG:
                    stage_a(t)
                # both js' score strips first: Act's exp queue stays a
                # half-tick ahead of the mm2 reads of the fresh et tiles
                for j in (2 * (t - 7) - 1, 2 * (t - 7)):
                    if 0 <= j < SB + 2:
                        jblock_mm1(j)
                for j in (2 * (t - 7) - 1, 2 * (t - 7)):
                    if 0 <= j < SB + 1:
                        jblock(j)

    nc.compile()
    return nc


def _prep_core_inputs(c, x, ve, cos, sin, Wq, Wk, Wv, Wo, Wg):
    b = c // TP
    h0 = (c % TP) * HD
    xT = np.ascontiguousarray(x[b].T).astype(BF_NP)          # [E, S]
    xg2 = np.empty((NG, 128, 8 * 256), BF_NP)
    for g in range(NG):
        for k8 in range(8):
            xg2[g, :, k8 * 256:(k8 + 1) * 256] = \
                xT[k8 * 128:(k8 + 1) * 128, g * 256:(g + 1) * 256]
    wg_pad = np.zeros((E, 1), np.float32)
    wg_pad[:GATE_CH, 0] = Wg[:, 0]
    wqkv = np.concatenate([Wq[:, h0:h0 + HD], Wk, Wv, wg_pad], axis=1)
    wqg = np.ascontiguousarray(
        wqkv.reshape(8, 128, QKW).transpose(1, 0, 2)
        .reshape(128, 8 * QKW)).astype(BF_NP)
    ccss = np.concatenate([cos, cos, sin, -sin], axis=1)     # [S, 128]
    cs2 = np.ascontiguousarray(
        ccss.reshape(SB, 128, 128).transpose(1, 0, 2).reshape(128, SB * 128)
    ).astype(BF_NP)
    ve2 = np.ascontiguousarray(
        (2.0 * ve[b]).reshape(SB, 128, 64).transpose(1, 0, 2)
        .reshape(128, SB * 64)).astype(BF_NP)
    wo2 = np.ascontiguousarray(
        Wo[h0:h0 + HD, :].reshape(2, 128, E)).astype(BF_NP)
    ii = np.arange(128)
    md = (ii[None, :] >= ii[:, None]).astype(np.float32)     # [ki, qi]
    mf = 1.0 - md
    mkd = np.tile(md, (1, 2)).reshape(128, 256).astype(BF_NP)
    mkf = np.tile(mf, (1, 2)).reshape(128, 256).astype(BF_NP)
    return dict(xg2=xg2, wqg=wqg, cs2=cs2, ve2=ve2, wo2=wo2,
                mkd=mkd, mkf=mkf)


def kernel(x, ve, cos, sin, Wq, Wk, Wv, Wo, Wg, window_size):
    assert int(window_size) == WIN
    x = np.asarray(x, np.float32)
    ve = np.asarray(ve, np.float32)
    cos = np.asarray(cos, np.float32)
    sin = np.asarray(sin, np.float32)
    Wq = np.asarray(Wq, np.float32)
    Wk = np.asarray(Wk, np.float32)
    Wv = np.asarray(Wv, np.float32)
    Wo = np.asarray(Wo, np.float32)
    Wg = np.asarray(Wg, np.float32)

    if _NC_CACHE[0] is None:
        _NC_CACHE[0] = _build()
    nc = _NC_CACHE[0]

    in_maps = [_prep_core_inputs(c, x, ve, cos, sin, Wq, Wk, Wv, Wo, Wg)
               for c in range(NCORES)]
    res = run_bass_kernel_spmd(nc, in_maps, core_ids=list(range(NCORES)),
                               trace=TRACE)
    LAST_RESULT[0] = res

    out = np.zeros((B, S, E), np.float32)
    for c in range(NCORES):
        out[c // TP] += res.results[c]["y"].astype(np.float32).reshape(S, E)
    return out


# revision 66
# speedup vs baseline: 1.0200x; 1.0085x over previous
"""Trainium2 Bass kernel for MQA sliding-window causal self-attention.

Sharding: 8 cores = DP(batch=2) x TP(head-groups=4). Each core computes 4 of
16 query heads for one batch element, shared KV head replicated. Host
pre-packs transposed/padded bf16 layouts (Wk pre-scaled by 1/8 so the
softmax scale folds into k's rmsnorm term); gathers + sums the 4 TP partial
outputs per batch element.

v2 vs baseline (152349ns):
  - ONE act table for the whole kernel: rstd = exp(-0.5*ln(ssum)) via the
    natural_log_exp_and_others set (act-table monkeypatch strips ln/exp from
    other sets so the greedy CFG pass can't thrash Sqrt<->Exp tables: was 11
    LoadActFuncSet = 14.1us on Act + PE stalls).
  - startup: wq + first x group split into 8 per-k8 DMA slices on two HWDGE
    queues so the first projection matmul issues at ~0.6us (was 7.9us gap).
  - stage_b PSUM->SBUF copies + y output copies moved Act->Pool (gpsimd);
    square moved Act->DVE (bf16 2x self-mul); Act runs ~only exp.
  - edge masks gpsimd->DVE (bf16 2x) and merged across the 2 head-pair sets
    (one et tile [128,2ps,2,1152] per j-block).
  - epilogue: softmax divide writes aoT halves directly (PSUM in0 allows
    out-partition-base offset), killing the aop intermediate, its copies,
    and the odd-half SBUF DMA.
"""
import numpy as np
import ml_dtypes
from contextlib import ExitStack

import concourse.bass as bass
import concourse.tile as tile
import concourse.mybir as mybir
from concourse import bacc
from concourse.bass_utils import run_bass_kernel_spmd

# ---- act-table selection patch: keep ln/exp ONLY in the one set that has
# both, so the greedy table-load pass emits a single LoadActFuncSet ----
import concourse.hw_specs as _hs
import concourse.bacc as _bacc_mod

_ONLY_SET = "natural_log_exp_and_others"
_orig_get_tables = _hs.get_activation_tables


def _patched_tables(arch):
    tabs = dict(_orig_get_tables(arch))
    keep = tabs[_ONLY_SET]
    return {k: (v if k == _ONLY_SET else (v - keep)) for k, v in tabs.items()}


_bacc_mod.get_activation_tables = _patched_tables

F32 = mybir.dt.float32
BF = mybir.dt.bfloat16
AF = mybir.ActivationFunctionType
ALU = mybir.AluOpType
BF_NP = ml_dtypes.bfloat16

B, S, E, H, KV, D = 2, 2048, 1024, 16, 1, 64
HALF = D // 2
GATE_CH = 32
WIN = 1024
NCORES = 8
TP = 4
HPC = H // TP            # heads per core = 4
HD = HPC * D             # per-core q width = 256
SB = S // 128            # 16 s-blocks
WB = WIN // 128          # 8 window blocks
QKW = HD + 2 * D + 1     # 385 (q 256 | k 64 | v 64 | gate 1)
RW = HD + D              # 320 roped width (4 q heads + k)
QNW = HD + 2 * D         # 384 qn width (q 256 | k | k-dup)
GW = 2                   # s-blocks per phase-1 group
NG = SB // GW            # 8 groups

TRACE = False
LAST_RESULT = [None]
_NC_CACHE = [None]


def _build():
    nc = bacc.Bacc()

    xg2 = nc.dram_tensor("xg2", [NG, 128, 8 * 256], BF, kind="ExternalInput")
    wqg = nc.dram_tensor("wqg", [128, 8 * QKW], BF, kind="ExternalInput")
    cs2 = nc.dram_tensor("cs2", [128, SB * 128], BF, kind="ExternalInput")
    ve2 = nc.dram_tensor("ve2", [128, SB * 64], BF, kind="ExternalInput")
    wo2 = nc.dram_tensor("wo2", [2, 128, E], BF, kind="ExternalInput")
    mkd = nc.dram_tensor("mkd", [128, 2 * 128], BF, kind="ExternalInput")
    mkf = nc.dram_tensor("mkf", [128, 2 * 128], BF, kind="ExternalInput")
    y = nc.dram_tensor("y", [SB, 128, E], BF, kind="ExternalOutput")

    with tile.TileContext(nc) as tc, ExitStack() as top:
        const = top.enter_context(tc.tile_pool(name="const", bufs=1))
        persist = top.enter_context(tc.tile_pool(name="persist", bufs=1))

        # ---- persistent activations ----
        qkT = [persist.tile([128, 3, GW * 128], BF, name=f"qkT{g}")
               for g in range(NG)]
        aoT = [persist.tile([128, S], BF, name=f"aoT{i}") for i in range(2)]
        vex = [persist.tile([128, 128], BF, name=f"vex{s}")
               for s in range(SB)]
        for s in range(SB):
            nc.gpsimd.memset(vex[s][:, 64:128], 1.0)
        # rstd per group: [:, :, 0:HPC] = q heads, [:, :, HPC] = k
        rg = [persist.tile([128, GW, 5], F32, name=f"rg{g}")
              for g in range(NG)]
        gate_sb = persist.tile([128, SB], F32)
        ge = persist.tile([128, SB], F32)
        gd = persist.tile([128, SB], F32)
        sig = persist.tile([128, SB], F32)

        maskd_sb = const.tile([128, 2, 128], BF)
        maskf_sb = const.tile([128, 2, 128], BF)
        cs_sb = const.tile([128, SB, 128], BF)
        ve_sb = const.tile([128, SB, 64], BF)
        wo_sb = [const.tile([128, E], BF, name=f"wo{i}") for i in range(2)]
        # weights in 2+2+4 k8-chunks: HWDGE costs ~625ns per DMA regardless
        # of size, so keep the count low, but quarter the first chunks so
        # the opening projection can issue ~1.5us sooner
        WQP = [(0, 2), (2, 2), (4, 4)]
        wq_h = [const.tile([128, n, QKW], BF, name=f"wqh{i}")
                for i, (k0, n) in enumerate(WQP)]

        with ExitStack() as p1:
            xpool = p1.enter_context(tc.tile_pool(name="xg", bufs=1))
            work = p1.enter_context(tc.tile_pool(name="work", bufs=1))
            big_psp = p1.enter_context(tc.tile_pool(name="big_ps", bufs=1,
                                                    space="PSUM"))
            strip_psp = p1.enter_context(tc.tile_pool(name="strip", bufs=1,
                                                      space="PSUM"))
            acc_psp = p1.enter_context(tc.tile_pool(name="acc", bufs=1,
                                                    space="PSUM"))
            expp = p1.enter_context(tc.tile_pool(name="expp", bufs=1))
            ep = p1.enter_context(tc.tile_pool(name="ep", bufs=1))
            yp = p1.enter_context(tc.tile_pool(name="yp", bufs=1))

            xg = {}
            st = {}

            # group-0 x chunks interleaved with the weight chunks on two
            # HWDGE queues
            xg0_h = [xpool.tile([128, n, 256], BF, tag=f"xg0{i}",
                                name=f"xg0h{i}", bufs=1)
                     for i, (k0, n) in enumerate(WQP)]
            for i, (k0, n) in enumerate(WQP):
                nc.sync.dma_start(
                    wq_h[i], wqg[:, k0 * QKW:(k0 + n) * QKW]
                    .rearrange("p (k c) -> p k c", k=n))
                nc.scalar.dma_start(
                    xg0_h[i], xg2[0, :, k0 * 256:(k0 + n) * 256]
                    .rearrange("p (k c) -> p k c", k=n))

            def wq_part(k8):
                i = 0 if k8 < 2 else (1 if k8 < 4 else 2)
                return wq_h[i][:, k8 - WQP[i][0], :]

            def xg0_part(k8):
                i = 0 if k8 < 2 else (1 if k8 < 4 else 2)
                return xg0_h[i][:, k8 - WQP[i][0], :]

            def load_group(g):
                t = xpool.tile([128, 8, 256], BF, tag="xg", name="xg_t", bufs=5)
                nc.sync.dma_start(
                    t, xg2[g, :, :].rearrange("p (k c) -> p k c", k=8))
                xg[g] = t

            load_group(1)
            # remaining constants on the Act HWDGE queue, behind nothing hot
            nc.scalar.dma_start(cs_sb,
                                cs2[:, :].rearrange("p (s c) -> p s c", s=SB))
            nc.scalar.dma_start(ve_sb,
                                ve2[:, :].rearrange("p (s c) -> p s c", s=SB))
            nc.scalar.dma_start(maskd_sb,
                                mkd[:, :].rearrange("p (h x) -> p h x", h=2))
            nc.scalar.dma_start(maskf_sb,
                                mkf[:, :].rearrange("p (h x) -> p h x", h=2))
            for i in range(2):
                nc.scalar.dma_start(wo_sb[i], wo2[i, :, :])

            def stage_a(g):
                # rotate the projection outputs through ALL THREE psum tags
                # (their slots are the same 2KB size; strips/acc are idle
                # until the attention loop) so the projections aren't
                # throttled by the 2-deep big-tag <-> stage_b round-trip
                r = g % 3
                if r == 1:
                    stp = strip_psp.tile([128, 2, 512], F32, tag="strip",
                                         name="strip", bufs=2)
                    ps_pair = [stp[:, li, 0:QKW] for li in range(GW)]
                elif r == 2:
                    ps_pair = []
                    for li in range(GW):
                        at = acc_psp.tile([128, 2, 256], F32, tag="acc",
                                          name="acc", bufs=2)
                        ps_pair.append(
                            at[:].rearrange("p a b -> p (a b)")[:, 0:QKW])
                else:
                    ps_pair = []
                    for li in range(GW):
                        big = big_psp.tile([128, 512], F32, tag="big",
                                           name="big_ps", bufs=2)
                        ps_pair.append(big[:, 0:QKW])
                for li in range(GW):
                    lcol = slice(li * 128, (li + 1) * 128)
                    for k8 in range(8):
                        xs = (xg0_part(k8)[:, lcol] if g == 0
                              else xg[g][:, k8, lcol])
                        nc.tensor.matmul(ps_pair[li], xs, wq_part(k8),
                                         start=(k8 == 0), stop=(k8 == 7),
                                         skip_group_check=True)
                st[g] = dict(ps=ps_pair)

            def stage_b(g):
                # one wide PSUM->SBUF copy per s-block (frees the big PSUM
                # buf sooner for the next projection); v/gate extracted from
                # SBUF on Pool, off the critical chain
                s_ = st[g]
                qkvsb = work.tile([128, GW, QKW], BF, tag="qkvsb",
                                  name="qkvsb", bufs=3)
                for li in range(GW):
                    ps_t = s_["ps"][li]
                    nc.scalar.copy(qkvsb[:, li, :], ps_t)
                for li in range(GW):
                    sb = g * GW + li
                    nc.gpsimd.tensor_copy(vex[sb][:, 0:64],
                                          qkvsb[:, li, RW:RW + 64])
                    nc.gpsimd.tensor_copy(gate_sb[:, sb:sb + 1],
                                          qkvsb[:, li, RW + 64:QKW])
                s_["qkvsb"] = qkvsb
                del s_["ps"]

            def csbc(g, off, width):
                # [128, li(2), 5-head bcast, width] view of cos/sin table
                return bass.AP(tensor=cs_sb.tensor,
                               offset=cs_sb.offset + (g * GW) * 128 + off,
                               ap=[list(cs_sb.ap[0]), [128, GW], [0, 5],
                                   [1, width]])

            def stage_c(g):
                s_ = st[g]
                qsb = s_["qkvsb"]
                qk5 = bass.AP(tensor=qsb.tensor, offset=qsb.offset,
                              ap=[list(qsb.ap[0]), [QKW, GW], [D, 5], [1, D]])
                tm1 = work.tile([128, GW, RW], BF, tag="tm1", name="tm1",
                                bufs=3)
                tm2 = work.tile([128, GW, RW], BF, tag="tm2", name="tm2",
                                bufs=3)
                tm1v = tm1[:].rearrange("p l (h d) -> p l h d", h=5)
                tm2v = tm2[:].rearrange("p l (h d) -> p l h d", h=5)
                nc.vector.tensor_mul(tm1v, qk5, csbc(g, 0, D))
                nc.gpsimd.tensor_mul(tm2v[:, :, :, 0:HALF],
                                     qk5[:, :, :, HALF:D], csbc(g, D, HALF))
                nc.vector.tensor_mul(tm2v[:, :, :, HALF:D],
                                     qk5[:, :, :, 0:HALF],
                                     csbc(g, D + HALF, HALF))
                qk_r = work.tile([128, GW, RW], BF, tag="qkr", name="qk_r",
                                 bufs=4)
                nc.vector.tensor_add(qk_r, tm1, tm2)
                s_["qk_r"] = qk_r

            def stage_d(g):
                s_ = st[g]
                qk_r = s_["qk_r"]
                sq = work.tile([128, GW, RW], BF, tag="sq", name="sq", bufs=3)
                # Act while the j-loop start hangs on this chain, DVE once
                # the exp stream owns Act
                if g <= 5:
                    nc.scalar.square(sq, qk_r)
                else:
                    nc.vector.tensor_mul(sq, qk_r, qk_r)
                ssum = work.tile([128, GW, 5], BF, tag="ssum", name="ssum",
                                 bufs=3)
                with nc.allow_low_precision(reason="rmsnorm ssum bf16: "
                                            "0.4% on ssum -> 0.2% on rstd"):
                    nc.vector.reduce_sum(
                        ssum[:].rearrange("p a b -> p (a b)"),
                        sq[:].rearrange("p l (h d) -> p (l h) d", h=5),
                        axis=mybir.AxisListType.X)
                # rstd_q = exp(-0.5*ln(ssum)); rstd_k = exp(-0.5*ln(ssum/64))
                # = sqrt(64/ssum) -- the D-scale enters via the Ln input
                # scale (rmsnorm is scale-invariant, so it can't come from a
                # host-side Wk scale)
                lt = work.tile([128, GW, 5], F32, tag="lt", name="lt", bufs=3)
                nc.scalar.activation(lt[:, :, 0:HPC], ssum[:, :, 0:HPC],
                                     AF.Ln, bias=0.0, scale=1.0)
                nc.scalar.activation(lt[:, :, HPC:5], ssum[:, :, HPC:5],
                                     AF.Ln, bias=0.0, scale=1.0 / D)
                nc.scalar.activation(rg[g][:], lt, AF.Exp, bias=0.0,
                                     scale=-0.5)

            def stage_e(g):
                s_ = st[g]
                qn = work.tile([128, GW, QNW], BF, tag="qn", name="qn", bufs=4)
                rbc = bass.AP(tensor=rg[g].tensor, offset=rg[g].offset,
                              ap=[list(rg[g].ap[0]), [5, GW], [1, HPC],
                                  [0, D]])
                nc.vector.tensor_mul(
                    qn[:, :, 0:HD].rearrange("p l (h d) -> p l h d", h=HPC),
                    s_["qk_r"][:, :, 0:HD].rearrange("p l (h d) -> p l h d",
                                                     h=HPC),
                    rbc)
                # roped k (unnormalized, pre-scaled 1/8), duplicated twice
                kin = bass.AP(tensor=s_["qk_r"].tensor,
                              offset=s_["qk_r"].offset + HD,
                              ap=[list(s_["qk_r"].ap[0]), [RW, GW], [0, 2],
                                  [1, D]])
                kout = bass.AP(tensor=qn.tensor, offset=qn.offset + HD,
                               ap=[list(qn.ap[0]), [QNW, GW], [D, 2], [1, D]])
                nc.vector.tensor_copy(kout, kin)
                s_["qn"] = qn

            def stage_f(g):
                s_ = st.pop(g)
                qn = s_["qn"]
                for li in range(GW):
                    lcol = slice(li * 128, (li + 1) * 128)
                    nc.sync.dma_start(qkT[g][:, :, lcol], qn[:, li, :],
                                      transpose=True)

            def gate_group(g):
                # per-group gate: spreads the sigmoid + vex gating over the
                # phase-1 ticks instead of a 16-op DVE burst at t=NG
                gcol = slice(g * GW, (g + 1) * GW)
                nc.scalar.activation(ge[:, gcol], gate_sb[:, gcol], AF.Exp,
                                     bias=0.0, scale=-1.0)
                nc.vector.tensor_scalar_add(gd[:, gcol], ge[:, gcol], 1.0)
                nc.vector.reciprocal(sig[:, gcol], gd[:, gcol])
                for sb in range(g * GW, (g + 1) * GW):
                    nc.vector.scalar_tensor_tensor(
                        out=vex[sb][:, 0:64], in0=ve_sb[:, sb, :],
                        scalar=sig[:, sb:sb + 1], in1=vex[sb][:, 0:64],
                        op0=ALU.mult, op1=ALU.add)

            # ---------------- attention j-step pieces ----------------
            exps = {0: {}, 1: {}}
            acst = {}

            def mm1(ps, j):
                nq = min(j + WB + 1, SB) - j
                et = expp.tile([128, 2, (WB + 1) * 128], BF, tag=f"exp{ps}",
                               name=f"exp{ps}", bufs=10)
                exps[ps][j] = et
                kt = qkT[j // 2]
                jcol = slice((j % 2) * 128, (j % 2) * 128 + 128)
                rk_ap = rg[j // 2][:, (j % 2), HPC:5]
                # group-aligned segments: [1] if j odd, then pairs, tail [1]
                segs = []
                b = j
                if b % 2 == 1:
                    segs.append((b, 1))
                    b += 1
                while b + 1 < j + nq:
                    segs.append((b, 2))
                    b += 2
                if b < j + nq:
                    segs.append((b, 1))
                # pack segments into strip buffers of <= 4 blocks, exp per buf
                si = 0
                off = 0
                while si < len(segs):
                    take = []
                    blk = 0
                    while si < len(segs) and blk + segs[si][1] <= 4:
                        take.append(segs[si])
                        blk += segs[si][1]
                        si += 1
                    stp = strip_psp.tile([128, 2, 512], F32, tag="strip",
                                         name="strip", bufs=2)
                    co = 0
                    for b0, nb in take:
                        w = nb * 128
                        g0 = b0 // 2
                        qcol = slice((b0 % 2) * 128, (b0 % 2) * 128 + w)
                        nc.tensor.matmul(stp[:, 0, co:co + w],
                                         kt[0:64, 2, jcol],
                                         qkT[g0][0:64, ps, qcol],
                                         start=True, stop=True,
                                         skip_group_check=True)
                        nc.tensor.matmul(stp[:, 1, co:co + w],
                                         kt[64:128, 2, jcol],
                                         qkT[g0][64:128, ps, qcol],
                                         start=True, stop=True,
                                         skip_group_check=True)
                        co += w
                    cw = blk * 128
                    nc.scalar.activation(et[:, :, off:off + cw],
                                         stp[:, :, 0:cw], AF.Exp,
                                         bias=0.0, scale=rk_ap)
                    if off == 0:
                        nc.vector.tensor_mul(et[:, :, 0:128], et[:, :, 0:128],
                                             maskd_sb)
                    off += cw
                if nq == WB + 1:
                    fcol = slice(WB * 128, (WB + 1) * 128)
                    nc.vector.tensor_mul(et[:, :, fcol], et[:, :, fcol],
                                         maskf_sb)

            def mm2pair(ps, m):
                q0, q1 = 2 * m, 2 * m + 1
                a = acc_psp.tile([128, 2, 256], F32, tag="acc", name="acc",
                                 bufs=2)
                first = True
                if q0 - WB >= 0:
                    jj = q0 - WB
                    o0 = (q0 - jj) * 128
                    nc.tensor.matmul(
                        a[:, :, 0:128], vex[jj][:],
                        exps[ps][jj][:, :, o0:o0 + 128],
                        start=True, stop=False, skip_group_check=True)
                    first = False
                for jj in range(max(0, q1 - WB), q0 + 1):
                    off = (q0 - jj) * 128
                    nc.tensor.matmul(a, vex[jj][:],
                                     exps[ps][jj][:, :, off:off + 256],
                                     start=first, stop=False,
                                     skip_group_check=True)
                    first = False
                nc.tensor.matmul(a[:, :, 128:256], vex[q1][:],
                                 exps[ps][q1][:, :, 0:128],
                                 start=False, stop=True,
                                 skip_group_check=True)
                acst[(ps, m)] = dict(a=a)

            def epi1(ps, m):
                s_ = acst[(ps, m)]
                rec = ep.tile([64, 2, 256], BF, tag=f"rec{ps}",
                              name=f"rec{ps}", bufs=2)
                with nc.allow_low_precision(reason="softmax denom recip"):
                    nc.vector.reciprocal(rec, s_["a"][64:128, :, :])
                s_["rec"] = rec

            def epi2(ps, m):
                s_ = acst.pop((ps, m))
                a, rec = s_["a"], s_["rec"]
                scol = slice(2 * m * 128, (2 * m + 2) * 128)
                # write aoT halves straight from PSUM x rec (PSUM in0 lets
                # the out partition base differ from the input bases)
                nc.vector.tensor_mul(aoT[ps][0:64, scol], a[0:64, 0, :],
                                     rec[:, 0, :])
                nc.vector.tensor_mul(aoT[ps][64:128, scol], a[0:64, 1, :],
                                     rec[:, 1, :])

            def mm3(sb, y_t, half, tail=False):
                scol = slice(sb * 128, (sb + 1) * 128)
                for nch in range(2):
                    y_ps = big_psp.tile([128, 512], F32, tag="big",
                                        name="y_ps", bufs=2)
                    for i in range(2):
                        nc.tensor.matmul(y_ps, aoT[i][:, scol],
                                         wo_sb[i][:, nch * 512:(nch + 1) * 512],
                                         start=(i == 0), stop=(i == 1),
                                         skip_group_check=True)
                    # tail: Act is idle after the last exp, so split the
                    # final copies across both engines to shorten the drain
                    eng = nc.scalar.copy if (tail and nch == 1) else \
                        nc.vector.tensor_copy
                    eng(y_t[:, half, nch * 512:(nch + 1) * 512], y_ps)

            yts = {}

            def jblock_mm1(j):
                if j < SB:
                    mm1(0, j)
                    mm1(1, j)

            def jblock(j):
                # mm3 for the pair m_ is split across this odd tick and the
                # following even one: 2 y_ps PSUM allocs per tick instead of
                # 4 keeps the DVE y-copy off mm3's big-tag rotation wait
                if j % 2 == 1:
                    if j >= 3 and (j - 3) // 2 < WB - 1:
                        m_ = (j - 3) // 2
                        for ps in range(2):
                            epi2(ps, m_)
                        y_t = yp.tile([128, 2, E], BF, tag="ysb", name="y_t",
                                      bufs=2)
                        yts[m_] = y_t
                        mm3(2 * m_, y_t, 0, tail=(m_ >= 5))
                    if (j - 1) // 2 < WB:
                        m = (j - 1) // 2
                        mm2pair(0, m)
                        mm2pair(1, m)
                        if m == WB - 1:
                            # final pair: recip right behind the last acc so
                            # the wind-down finishes a tick earlier
                            epi1(0, m)
                            epi1(1, m)
                else:
                    if j >= 4 and (j - 4) // 2 < WB - 1:
                        m_ = (j - 4) // 2
                        y_t = yts.pop(m_)
                        mm3(2 * m_ + 1, y_t, 1, tail=(m_ >= 3))
                        nc.sync.dma_start(
                            y[2 * m_:2 * m_ + 2, :, :]
                            .rearrange("s p e -> p s e"), y_t)
                    if j >= 2 and j // 2 - 1 < WB - 1:
                        for ps in range(2):
                            epi1(ps, j // 2 - 1)
                    if j == 2 * WB:
                        # final pair: epi2 + both mm3 halves + split DMAs in
                        # this tick instead of spilling into j=17
                        m_ = WB - 1
                        for ps in range(2):
                            epi2(ps, m_)
                        y_t = yp.tile([128, 2, E], BF, tag="ysb", name="y_t",
                                      bufs=2)
                        mm3(2 * m_, y_t, 0, tail=True)
                        nc.sync.dma_start(
                            y[2 * m_:2 * m_ + 1, :, :]
                            .rearrange("s p e -> p s e"), y_t[:, 0:1, :])
                        mm3(2 * m_ + 1, y_t, 1, tail=True)
                        nc.sync.dma_start(
                            y[2 * m_ + 1:2 * m_ + 2, :, :]
                            .rearrange("s p e -> p s e"), y_t[:, 1:2, :])

            # ---------------- merged tick loop ----------------
            for t in range(18):
                if t + 2 < NG:
                    load_group(t + 2)
                if 0 <= t - 3 < NG:
                    stage_f(t - 3)
                if 0 <= t - 1 < NG:
                    stage_b(t - 1)
                if 0 <= t - 2 < NG:
                    stage_d(t - 2)
                    stage_e(t - 2)
                if 0 <= t - 1 < NG:
                    stage_c(t - 1)
                if 0 <= t - 3 < NG:
                    gate_group(t - 3)
                if t < NG:
                    stage_a(t)
                # both js' score strips first: Act's exp queue stays a
                # half-tick ahead of the mm2 reads of the fresh et tiles
                for j in (2 * (t - 7) - 1, 2 * (t - 7)):
                    if 0 <= j < SB + 2:
                        jblock_mm1(j)
                for j in (2 * (t - 7) - 1, 2 * (t - 7)):
                    if 0 <= j < SB + 1:
                        jblock(j)

    nc.compile()
    return nc


def _prep_core_inputs(c, x, ve, cos, sin, Wq, Wk, Wv, Wo, Wg):
    b = c // TP
    h0 = (c % TP) * HD
    xT = np.ascontiguousarray(x[b].T).astype(BF_NP)          # [E, S]
    xg2 = np.empty((NG, 128, 8 * 256), BF_NP)
    for g in range(NG):
        for k8 in range(8):
            xg2[g, :, k8 * 256:(k8 + 1) * 256] = \
                xT[k8 * 128:(k8 + 1) * 128, g * 256:(g + 1) * 256]
    wg_pad = np.zeros((E, 1), np.float32)
    wg_pad[:GATE_CH, 0] = Wg[:, 0]
    wqkv = np.concatenate([Wq[:, h0:h0 + HD], Wk, Wv, wg_pad], axis=1)
    wqg = np.ascontiguousarray(
        wqkv.reshape(8, 128, QKW).transpose(1, 0, 2)
        .reshape(128, 8 * QKW)).astype(BF_NP)
    ccss = np.concatenate([cos, cos, sin, -sin], axis=1)     # [S, 128]
    cs2 = np.ascontiguousarray(
        ccss.reshape(SB, 128, 128).transpose(1, 0, 2).reshape(128, SB * 128)
    ).astype(BF_NP)
    ve2 = np.ascontiguousarray(
        (2.0 * ve[b]).reshape(SB, 128, 64).transpose(1, 0, 2)
        .reshape(128, SB * 64)).astype(BF_NP)
    wo2 = np.ascontiguousarray(
        Wo[h0:h0 + HD, :].reshape(2, 128, E)).astype(BF_NP)
    ii = np.arange(128)
    md = (ii[None, :] >= ii[:, None]).astype(np.float32)     # [ki, qi]
    mf = 1.0 - md
    mkd = np.tile(md, (1, 2)).reshape(128, 256).astype(BF_NP)
    mkf = np.tile(mf, (1, 2)).reshape(128, 256).astype(BF_NP)
    return dict(xg2=xg2, wqg=wqg, cs2=cs2, ve2=ve2, wo2=wo2,
                mkd=mkd, mkf=mkf)


def kernel(x, ve, cos, sin, Wq, Wk, Wv, Wo, Wg, window_size):
    assert int(window_size) == WIN
    x = np.asarray(x, np.float32)
    ve = np.asarray(ve, np.float32)
    cos = np.asarray(cos, np.float32)
    sin = np.asarray(sin, np.float32)
    Wq = np.asarray(Wq, np.float32)
    Wk = np.asarray(Wk, np.float32)
    Wv = np.asarray(Wv, np.float32)
    Wo = np.asarray(Wo, np.float32)
    Wg = np.asarray(Wg, np.float32)

    if _NC_CACHE[0] is None:
        _NC_CACHE[0] = _build()
    nc = _NC_CACHE[0]

    in_maps = [_prep_core_inputs(c, x, ve, cos, sin, Wq, Wk, Wv, Wo, Wg)
               for c in range(NCORES)]
    res = run_bass_kernel_spmd(nc, in_maps, core_ids=list(range(NCORES)),
                               trace=TRACE)
    LAST_RESULT[0] = res

    out = np.zeros((B, S, E), np.float32)
    for c in range(NCORES):
        out[c // TP] += res.results[c]["y"].astype(np.float32).reshape(S, E)
    return out
